# revision 1
# baseline (speedup 1.0000x reference)
"""Trainium2 Bass kernel for nn_Encoder_Postnet (ragged_sequence).

Computation (reference):
    idx   = sequential aligner scan over (align_phone, text_phone)   [B,T]
    out   = enc[idx] + pitch @ w_pitch + b_pitch + emb_beats[beats]
            + (enc[idx] + pe) @ w_pos + b_pos

Key algebraic restructure: the duration-expansion gather commutes with the
E x E linear, so
    out[t] = encG[idx_t] + (pe@w_pos + bias)[t] + pitch[t]*w_pitch + beats[t]*demb
with encG = enc @ (I + w_pos) computed once per batch row ([P,E] not [T,E]),
collapsing the big [B*T,E]@[E,E] matmul 8x and making the kernel memory-bound.

Sharding: pure data parallel, 2 batch rows per core across 8 cores.

Fast path (the uniform duration-8 expansion this model's inputs produce,
idx == arange(T)//8 for every row):
  phase A: encG = enc_row @ (I+w_pos) on PE (fp32); the result stays resident
           in SBUF split into bf16 hi/lo pairs (hi+lo keeps ~17 mantissa
           bits, and bf16 matmuls run 4x faster than fp32 on the PE).
  phase B: per 128-token group, one PSUM accumulation group of 4 bf16
           matmuls: S_j-one-hot expansion of encG rows (hi+lo) + identity
           matmuls adding the (pe@w_pos + bias) tile (hi+lo).  Then the
           pitch rank-1 term rides the DVE op that drains PSUM->SBUF
           (scalar_tensor_tensor), and the beats term runs on the otherwise
           idle GPSIMD.  The only DMA traffic is inputs-once + outputs-once.

General path (arbitrary idx): per-128-token indirect-DMA row gathers
(production-shaped offset [128,1] DynamicAP descriptors) + K=3 stream matmul.

The aligner scan itself is index metadata ([B,T] int32); it is computed on
host with a run-compressed O(B*P) algorithm exactly equivalent to the
reference recurrence, then consumed either as a uniformity proof (fast path)
or as gather offsets (general path).
"""

import sys

for _p in ("/opt/trn_rl_repo",):
    if _p not in sys.path:
        sys.path.insert(0, _p)

import numpy as np

B, P, T, E = 16, 1024, 8192, 256
NCORES = 8
RPC = B // NCORES          # batch rows per core
NGRP = T // 128            # 64 groups of 128 tokens per row
NSUP = 8                   # groups per super-chunk
DUR = T // P               # uniform duration of the fast path (8)
NW = 128 // DUR            # encG rows per group (16)

FORCE_GENERAL = False      # test hook: force the arbitrary-idx path
_CACHE = {}


# --------------------------------------------------------------------------
# Host: aligner index computation (exact replica of the reference recurrence)
# --------------------------------------------------------------------------
def compute_idx(align, text):
    """idx[b,0]=0; idx[b,j] = idx[b,j-1] if align[b,j]==text[b,idx[b,j-1]]
    else min(idx[b,j-1]+1, P-1).   Vectorized over batch via segment starts:
    the pointer advances i->i+1 at s_{i+1} = first j >= s_i+1 with
    align[j] != text[i]; within a run of align values equal to text[i] the
    first mismatch is the run end."""
    align = np.asarray(align)
    text = np.asarray(text)
    Bn, Tn = align.shape
    Pn = text.shape[1]
    diff = align[:, 1:] != align[:, :-1]                       # [B, T-1]
    c = np.full((Bn, Tn), Tn, np.int64)
    c[:, :-1] = np.where(diff, np.arange(1, Tn)[None, :], Tn)
    re = np.flip(np.minimum.accumulate(np.flip(c, axis=1), axis=1), axis=1)

    s = np.full((Bn, Pn), Tn, np.int64)
    s[:, 0] = 0
    cur = np.zeros(Bn, np.int64)
    arB = np.arange(Bn)
    for i in range(Pn - 1):
        j0 = cur + 1
        active = j0 < Tn
        j0c = np.minimum(j0, Tn - 1)
        eq = (align[arB, j0c] == text[:, i]) & active
        nxt = np.where(active, np.where(eq, re[arB, j0c], j0), Tn)
        s[:, i + 1] = nxt
        cur = nxt
    idx = np.empty((Bn, Tn), np.int32)
    pos = np.arange(Tn)
    for b in range(Bn):
        idx[b] = (np.searchsorted(s[b], pos, side="right") - 1).astype(np.int32)
    return idx


def _positional_encoding_f64(t, e):
    pos = np.arange(t, dtype=np.float64)[:, None]
    div = np.exp(np.arange(0, e, 2, dtype=np.float64) * (-np.log(10000.0) / e))
    ang = pos * div[None, :]
    return np.stack([np.sin(ang), np.cos(ang)], axis=-1).reshape(t, e)


def _bf16_split(x):
    import ml_dtypes
    x = np.asarray(x, np.float32)
    hi = x.astype(ml_dtypes.bfloat16)
    lo = (x - hi.astype(np.float32)).astype(ml_dtypes.bfloat16)
    return hi, lo


# --------------------------------------------------------------------------
# Device programs
# --------------------------------------------------------------------------
def build_nc_fast():
    from contextlib import ExitStack
    import concourse.tile as tile
    from concourse import bacc, mybir
    from concourse._compat import get_trn_type

    f32 = mybir.dt.float32
    bf16 = mybir.dt.bfloat16
    mult = mybir.AluOpType.mult
    add = mybir.AluOpType.add

    nc = bacc.Bacc(get_trn_type() or "TRN2", target_bir_lowering=False, debug=False)
    enc_hi = nc.declare_dram_parameter("enc_hi", [RPC, E, P], bf16, isOutput=False)
    enc_lo = nc.declare_dram_parameter("enc_lo", [RPC, E, P], bf16, isOutput=False)
    g_hi = nc.declare_dram_parameter("g_hi", [E, E], bf16, isOutput=False)
    g_lo = nc.declare_dram_parameter("g_lo", [E, E], bf16, isOutput=False)
    pe_w = nc.declare_dram_parameter("pe_w", [128, NGRP, E], f32, isOutput=False)
    sj_d = nc.declare_dram_parameter("sj", [128, NSUP * 128], bf16, isOutput=False)
    # stream-term lhsT rows banked at 32-partition strides (4 token-chunks)
    # to keep the per-partition footprint small for the DMA
    l5_d = nc.declare_dram_parameter("l5", [RPC, 128, T // 4], bf16, isOutput=False)
    w5_d = nc.declare_dram_parameter("w5", [128, E], bf16, isOutput=False)
    out = nc.declare_dram_parameter("out", [RPC, T, E], f32, isOutput=True)

    with tile.TileContext(nc) as tc, ExitStack() as ctx:
        const = ctx.enter_context(tc.tile_pool(name="const", bufs=1))
        pe_pool = ctx.enter_context(tc.tile_pool(name="pe", bufs=8))
        out_pool = ctx.enter_context(tc.tile_pool(name="outp", bufs=4))

        sj_sb = const.tile([128, NSUP * 128], bf16, tag="sj")
        nc.sync.dma_start(sj_sb[:], sj_d[:])
        w5_sb = const.tile([128, E], bf16, tag="w5")
        nc.sync.dma_start(w5_sb[:], w5_d[:])
        l5_sb, egh_keep, egl_keep = [], [], []
        for r in range(RPC):
            l5t = const.tile([128, T // 4], bf16, tag=f"l5_{r}")
            for cb in range(4):
                nc.gpsimd.dma_start(
                    l5t[32 * cb:32 * cb + 5, :], l5_d[r, 32 * cb:32 * cb + 5, :]
                )
            l5_sb.append(l5t)
            egh_keep.append(
                const.tile([128, NSUP, E], bf16, tag=f"egh{r}", name=f"egh{r}")
            )
            egl_keep.append(
                const.tile([128, NSUP, E], bf16, tag=f"egl{r}", name=f"egl{r}")
            )

        # ---- phase A: encG = enc @ (I+w_pos) as 3-term bf16-split matmuls;
        # result kept in SBUF as bf16 hi/lo.  psum layout [128p, m, e] ==
        # keep layout: row m*128+p at (partition p, block m).
        gh0 = const.tile([128, E], bf16, tag="gh0", name="gh0")
        gh1 = const.tile([128, E], bf16, tag="gh1", name="gh1")
        gl0 = const.tile([128, E], bf16, tag="gl0", name="gl0")
        gl1 = const.tile([128, E], bf16, tag="gl1", name="gl1")
        nc.sync.dma_start(gh0[:], g_hi[0:128, :])
        nc.sync.dma_start(gh1[:], g_hi[128:256, :])
        nc.sync.dma_start(gl0[:], g_lo[0:128, :])
        nc.sync.dma_start(gl1[:], g_lo[128:256, :])
        psum_a = ctx.enter_context(tc.tile_pool(name="psumA", bufs=1, space="PSUM"))
        psum_b = ctx.enter_context(tc.tile_pool(name="psumB", bufs=6, space="PSUM"))
        with tc.tile_pool(name="encT", bufs=2) as encT_pool:
            for r in range(RPC):
                eh0 = encT_pool.tile([128, P], bf16, tag="eh0")
                eh1 = encT_pool.tile([128, P], bf16, tag="eh1")
                el0 = encT_pool.tile([128, P], bf16, tag="el0")
                el1 = encT_pool.tile([128, P], bf16, tag="el1")
                nc.sync.dma_start(eh0[:], enc_hi[r, 0:128, :])
                nc.sync.dma_start(eh1[:], enc_hi[r, 128:256, :])
                nc.sync.dma_start(el0[:], enc_lo[r, 0:128, :])
                nc.sync.dma_start(el1[:], enc_lo[r, 128:256, :])
                for mh in range(2):
                    ps = psum_a.tile([128, 4 * E], f32, tag="psA", name="psA")
                    for mi in range(4):
                        m = mh * 4 + mi
                        sl = slice(m * 128, (m + 1) * 128)
                        terms = [
                            (eh0, gh0, True, False), (eh1, gh1, False, False),
                            (el0, gh0, False, False), (el1, gh1, False, False),
                            (eh0, gl0, False, False), (eh1, gl1, False, True),
                        ]
                        for lt, gt_, st, sp in terms:
                            nc.tensor.matmul(
                                ps[:, mi * E:(mi + 1) * E],
                                lhsT=lt[:, sl], rhs=gt_[:], start=st, stop=sp,
                            )
                        # drain per m-chunk so phase B super-chunk m can start
                        # before the rest of phase A finishes
                        hi = egh_keep[r][:, m, :]
                        nc.vector.tensor_copy(hi, ps[:, mi * E:(mi + 1) * E])
                        nc.vector.scalar_tensor_tensor(
                            out=egl_keep[r][:, m, :], in0=hi, scalar=-1.0,
                            in1=ps[:, mi * E:(mi + 1) * E], op0=mult, op1=add,
                        )

        # ---- phase B: per group, 5 bf16 matmuls into one PSUM group:
        # expansion hi/lo + pe hi/lo + the K=5 rank-1 stream matmul
        # (pitch_hi/lo x w_pitch_hi/lo cross terms + beats x demb_hi/lo).
        if True:
            for s in range(T // (NSUP * 128)):
                pe_t = pe_pool.tile([128, NSUP, E], f32, tag="pe")
                nc.sync.dma_start(pe_t[:], pe_w[:, s * NSUP:(s + 1) * NSUP, :])
                for r in range(RPC):
                    ot = out_pool.tile([128, NSUP, E], f32, tag="ot")
                    for j in range(NSUP):
                        g = s * NSUP + j
                        ps = psum_b.tile([128, E], f32, tag="ps")
                        sj_ap = sj_sb[:, j * 128:(j + 1) * 128]
                        nc.tensor.matmul(
                            ps[:], lhsT=sj_ap, rhs=egh_keep[r][:, s, :],
                            start=True, stop=False,
                        )
                        nc.tensor.matmul(
                            ps[:], lhsT=sj_ap, rhs=egl_keep[r][:, s, :],
                            start=False, stop=False,
                        )
                        cb = g // (NGRP // 4)          # token-chunk bank
                        u0 = (g % (NGRP // 4)) * 128
                        nc.tensor.matmul(
                            ps[:],
                            lhsT=l5_sb[r][32 * cb:32 * cb + 5, u0:u0 + 128],
                            rhs=w5_sb[32 * cb:32 * cb + 5, :],
                            start=False, stop=True,
                            tile_position=(32 * cb, 0),
                        )
                        # drain PSUM -> SBUF fused with the (pe@w_pos+bias) add
                        nc.vector.tensor_add(ot[:, j, :], ps[:], pe_t[:, j, :])
                    # out-stores go via the ACT HWDGE queue so they don't
                    # head-of-line-block SP's input loads; two half-stores so
                    # the first half streams while the second half computes
                    for h in range(2):
                        t0 = (s * NSUP + h * (NSUP // 2)) * 128
                        nc.scalar.dma_start(
                            out[r, t0:t0 + (NSUP // 2) * 128, :].rearrange(
                                "(n p) e -> p n e", p=128
                            ),
                            ot[:, h * (NSUP // 2):(h + 1) * (NSUP // 2), :],
                        )
    nc.compile()
    return nc


def build_nc_general():
    """Arbitrary-idx path: per-128-token indirect row gathers."""
    import concourse.bass as bass
    from contextlib import ExitStack
    import concourse.tile as tile
    from concourse import bacc, mybir
    from concourse._compat import get_trn_type

    f32 = mybir.dt.float32
    i32 = mybir.dt.int32

    nc = bacc.Bacc(get_trn_type() or "TRN2", target_bir_lowering=False, debug=False)
    enc_t = nc.declare_dram_parameter("enc_t", [RPC, E, P], f32, isOutput=False)
    g_mat = nc.declare_dram_parameter("g_mat", [E, E], f32, isOutput=False)
    pe_w = nc.declare_dram_parameter("pe_w", [128, NGRP, E], f32, isOutput=False)
    p3 = nc.declare_dram_parameter("p3", [RPC, 3, T], f32, isOutput=False)
    w3 = nc.declare_dram_parameter("w3", [3, E], f32, isOutput=False)
    idxo = nc.declare_dram_parameter(
        "idxo", [RPC, 128, NGRP], i32, isOutput=False
    )
    out = nc.declare_dram_parameter("out", [RPC, T, E], f32, isOutput=True)
    encg = nc.dram_tensor("encg", [RPC, P, E], f32)

    with tile.TileContext(nc) as tc, ExitStack() as ctx:
        const = ctx.enter_context(tc.tile_pool(name="const", bufs=1))
        encT_pool = ctx.enter_context(tc.tile_pool(name="encT", bufs=2))
        psum_pool = ctx.enter_context(tc.tile_pool(name="psum", bufs=2, space="PSUM"))
        eg_pool = ctx.enter_context(tc.tile_pool(name="eg", bufs=2))
        pe_pool = ctx.enter_context(tc.tile_pool(name="pe", bufs=2))
        gath_pool = ctx.enter_context(tc.tile_pool(name="gath", bufs=3))

        g0 = const.tile([128, E], f32, tag="g0")
        g1 = const.tile([128, E], f32, tag="g1")
        nc.sync.dma_start(g0[:], g_mat[0:128, :])
        nc.sync.dma_start(g1[:], g_mat[128:256, :])
        w3_sb = const.tile([3, E], f32, tag="w3")
        nc.sync.dma_start(w3_sb[:], w3[:, :])
        p3_sb = []
        ixo_sb = []
        for r in range(RPC):
            p3t = const.tile([3, T], f32, tag=f"p3_{r}")
            nc.sync.dma_start(p3t[:], p3[r])
            p3_sb.append(p3t)
            ixt = const.tile([128, NGRP], i32, tag=f"ixo_{r}")
            nc.sync.dma_start(ixt[:], idxo[r])
            ixo_sb.append(ixt)

        for r in range(RPC):
            et0 = encT_pool.tile([128, P], f32, tag="et0")
            et1 = encT_pool.tile([128, P], f32, tag="et1")
            nc.sync.dma_start(et0[:], enc_t[r, 0:128, :])
            nc.sync.dma_start(et1[:], enc_t[r, 128:256, :])
            ps = psum_pool.tile([128, 8 * E], f32, tag="ps")
            for m in range(8):
                nc.tensor.matmul(
                    ps[:, m * E:(m + 1) * E],
                    lhsT=et0[:, m * 128:(m + 1) * 128],
                    rhs=g0[:], start=True, stop=False,
                )
                nc.tensor.matmul(
                    ps[:, m * E:(m + 1) * E],
                    lhsT=et1[:, m * 128:(m + 1) * 128],
                    rhs=g1[:], start=False, stop=True,
                )
            eg = eg_pool.tile([128, 8 * E], f32, tag="eg")
            nc.vector.tensor_copy(eg[:], ps[:])
            nc.sync.dma_start(
                encg[r].rearrange("(m p) e -> p m e", p=128),
                eg[:].rearrange("q (m e) -> q m e", e=E),
            )

        encg_flat = encg[:].rearrange("r p e -> (r p) e")
        for s in range(T // (NSUP * 128)):
            pe_t = pe_pool.tile([128, NSUP, E], f32, tag="pe")
            nc.sync.dma_start(pe_t[:], pe_w[:, s * NSUP:(s + 1) * NSUP, :])
            for r in range(RPC):
                gt = gath_pool.tile([128, NSUP, E], f32, tag="gt")
                for g in range(NSUP):
                    gi = s * NSUP + g
                    nc.gpsimd.indirect_dma_start(
                        out=gt[:, g, :],
                        out_offset=None,
                        in_=encg_flat,
                        in_offset=bass.IndirectOffsetOnAxis(
                            ap=ixo_sb[r][:, gi:gi + 1], axis=0
                        ),
                    )
                nc.vector.tensor_add(gt[:], gt[:], pe_t[:])
                ps = psum_pool.tile([128, 8 * E], f32, tag="ps")
                for g in range(NSUP):
                    gi = s * NSUP + g
                    nc.tensor.matmul(
                        ps[:, g * E:(g + 1) * E],
                        lhsT=p3_sb[r][:, gi * 128:(gi + 1) * 128],
                        rhs=w3_sb[:],
                        start=True, stop=True,
                    )
                nc.vector.tensor_add(
                    gt[:], gt[:], ps[:].rearrange("q (n e) -> q n e", e=E)
                )
                nc.sync.dma_start(
                    out[r, s * NSUP * 128:(s + 1) * NSUP * 128, :].rearrange(
                        "(n p) e -> p n e", p=128
                    ),
                    gt[:],
                )
    nc.compile()
    return nc


def get_nc(fast):
    key = "nc_fast" if fast else "nc_gen"
    if key not in _CACHE:
        _CACHE[key] = build_nc_fast() if fast else build_nc_general()
    return _CACHE[key]


# --------------------------------------------------------------------------
# Host wrapper
# --------------------------------------------------------------------------
def make_in_maps(encoder_out, align_phone, text_phone, pitch, beats,
                 w_pitch, b_pitch, emb_beats, w_pos, b_pos):
    import ml_dtypes

    encoder_out = np.asarray(encoder_out, np.float32)
    pitch = np.asarray(pitch, np.float32)
    beats = np.asarray(beats)
    w_pitch = np.asarray(w_pitch, np.float32)
    w_pos = np.asarray(w_pos, np.float32)

    idx = compute_idx(np.asarray(align_phone), np.asarray(text_phone))
    fast = bool(np.all(idx == (np.arange(T, dtype=np.int32) // DUR)[None, :]))
    if FORCE_GENERAL:
        fast = False

    g_mat = (np.eye(E, dtype=np.float64) + w_pos.astype(np.float64)).astype(np.float32)
    pe = _positional_encoding_f64(T, E)
    pe_proj = pe @ w_pos.astype(np.float64)                          # [T, E]
    bias = (np.asarray(emb_beats[0], np.float64)
            + np.asarray(b_pitch, np.float64)
            + np.asarray(b_pos, np.float64))
    demb = (np.asarray(emb_beats[1], np.float64)
            - np.asarray(emb_beats[0], np.float64)).astype(np.float32)

    if fast:
        pe_tot = (pe_proj + bias[None, :]).astype(np.float32)
        pe_wrap = np.ascontiguousarray(pe_tot.reshape(NGRP, 128, E).swapaxes(0, 1))
        # S_j[k, t'] = 1 iff k == j*16 + t'//8
        rows = np.arange(128)[:, None]
        sj = np.concatenate(
            [(rows == (j * NW + np.arange(128) // DUR)[None, :]) for j in range(NSUP)],
            axis=1,
        ).astype(ml_dtypes.bfloat16)
        g_hi, g_lo = _bf16_split(g_mat)
        wp_hi, wp_lo = _bf16_split(w_pitch[0])
        db_hi, db_lo = _bf16_split(demb)
        w5_rows = np.stack([
            wp_hi, wp_lo, wp_hi,
            db_hi, db_lo,
        ]).astype(ml_dtypes.bfloat16)
        w5 = np.zeros((128, E), ml_dtypes.bfloat16)
        for cb in range(4):
            w5[32 * cb:32 * cb + 5] = w5_rows
        fast_common = {
            "pe_w": pe_wrap, "sj": sj,
            "g_hi": g_hi, "g_lo": g_lo, "w5": w5,
        }
    else:
        w3 = np.stack(
            [w_pitch[0].astype(np.float64), demb.astype(np.float64), bias]
        ).astype(np.float32)
        pe_wl = np.ascontiguousarray(
            pe_proj.astype(np.float32).reshape(NGRP, 128, E).swapaxes(0, 1)
        )

    in_maps = []
    for core in range(NCORES):
        rows_ = range(core * RPC, (core + 1) * RPC)
        enc_t = np.ascontiguousarray(
            encoder_out[core * RPC:(core + 1) * RPC].transpose(0, 2, 1)
        )
        if fast:
            import ml_dtypes as _md
            enc_hi, enc_lo = _bf16_split(enc_t)
            l5 = np.zeros((RPC, 128, T // 4), _md.bfloat16)
            for j, b in enumerate(rows_):
                p_hi, p_lo = _bf16_split(pitch[b, :, 0])
                bt = beats[b, :, 0].astype(_md.bfloat16)
                rows5 = np.stack([p_hi, p_hi, p_lo, bt, bt])     # [5, T]
                for cb in range(4):
                    l5[j, 32 * cb:32 * cb + 5] = rows5[
                        :, cb * (T // 4):(cb + 1) * (T // 4)
                    ]
            m = {"enc_hi": enc_hi, "enc_lo": enc_lo, "l5": l5, **fast_common}
        else:
            p3 = np.empty((RPC, 3, T), np.float32)
            idxo = np.empty((RPC, 128, NGRP), np.int32)
            for j, b in enumerate(rows_):
                p3[j, 0] = pitch[b, :, 0]
                p3[j, 1] = beats[b, :, 0].astype(np.float32)
                p3[j, 2] = 1.0
                idxo[j] = idx[b].reshape(NGRP, 128).T + j * P
            m = {"enc_t": enc_t, "g_mat": g_mat, "pe_w": pe_wl, "p3": p3,
                 "w3": w3, "idxo": idxo}
        in_maps.append(m)
    return fast, in_maps


def kernel(**inputs):
    from concourse.bass_utils import run_bass_kernel_spmd

    fast, in_maps = make_in_maps(**inputs)
    nc = get_nc(fast)
    res = run_bass_kernel_spmd(nc, in_maps, core_ids=list(range(NCORES)))
    out = np.concatenate([res.results[i]["out"] for i in range(NCORES)], axis=0)
    return np.ascontiguousarray(out.astype(np.float32))



# revision 3
# speedup vs baseline: 1.0600x; 1.0600x over previous
"""Trainium2 Bass kernel for nn_Encoder_Postnet (ragged_sequence).

Computation (reference):
    idx   = sequential aligner scan over (align_phone, text_phone)   [B,T]
    out   = enc[idx] + pitch @ w_pitch + b_pitch + emb_beats[beats]
            + (enc[idx] + pe) @ w_pos + b_pos

Key algebraic restructure: the duration-expansion gather commutes with the
E x E linear, so
    out[t] = encG[idx_t] + (pe@w_pos + bias)[t] + pitch[t]*w_pitch + beats[t]*demb
with encG = enc @ (I + w_pos) computed once per batch row ([P,E] not [T,E]),
collapsing the big [B*T,E]@[E,E] matmul 8x and making the kernel memory-bound.

Sharding: pure data parallel, 2 batch rows per core across 8 cores.

Fast path (the uniform duration-8 expansion this model's inputs produce,
idx == arange(T)//8 for every row).  The whole dataflow is bf16 (the harness
tolerance is 2e-2 relmax; bf16 roundoff is ~4e-3), which halves every DMA
stream vs f32 -- the cost model serializes all DMA traffic at 360 GB/s so
bytes moved is the wall clock:
  phase A: encG = enc_bf16 @ (I+w_pos) on PE (g kept as bf16 hi+lo for
           accuracy); psum drained by the ACT engine to a resident bf16 tile.
  phase B: per 128-token group, one K=32 one-hot expansion matmul (selects
           and repeats the 16 encG rows for this group) + one K=4 stream
           matmul (pitch x w_pitch hi/lo + beats x demb hi/lo) into PSUM.
           Per 4-group super-chunk the psum is drained with the
           (pe@w_pos + bias) bf16 tile added in:
             row 0: DVE fused tensor_add (psum + pe -> bf16 out tile)
             row 1: ACT copy (psum -> bf16) then DVE 2x-mode bf16 add of pe
           which balances DVE/ACT busy time under the DMA roofline.
  Output is stored as bf16 (converted to f32 on host).

General path (arbitrary idx): per-128-token indirect-DMA row gathers
(production-shaped offset [128,1] DynamicAP descriptors) + K=3 stream matmul.

The aligner scan itself is index metadata ([B,T] int32); it is computed on
host with a run-compressed O(B*P) algorithm exactly equivalent to the
reference recurrence, then consumed either as a uniformity proof (fast path)
or as gather offsets (general path).
"""

import sys

for _p in ("/opt/trn_rl_repo",):
    if _p not in sys.path:
        sys.path.insert(0, _p)

import numpy as np

B, P, T, E = 16, 1024, 8192, 256
NCORES = 8
RPC = B // NCORES          # batch rows per core
NGRP = T // 128            # 64 groups of 128 tokens per row
NSUP = 8                   # groups per super-chunk (general path)
DUR = T // P               # uniform duration of the fast path (8)
NW = 128 // DUR            # encG rows per group (16)

FORCE_GENERAL = False      # test hook: force the arbitrary-idx path
_CACHE = {}


# --------------------------------------------------------------------------
# Host: aligner index computation (exact replica of the reference recurrence)
# --------------------------------------------------------------------------
def compute_idx(align, text):
    """idx[b,0]=0; idx[b,j] = idx[b,j-1] if align[b,j]==text[b,idx[b,j-1]]
    else min(idx[b,j-1]+1, P-1).   Vectorized over batch via segment starts:
    the pointer advances i->i+1 at s_{i+1} = first j >= s_i+1 with
    align[j] != text[i]; within a run of align values equal to text[i] the
    first mismatch is the run end."""
    align = np.asarray(align)
    text = np.asarray(text)
    Bn, Tn = align.shape
    Pn = text.shape[1]
    diff = align[:, 1:] != align[:, :-1]                       # [B, T-1]
    c = np.full((Bn, Tn), Tn, np.int64)
    c[:, :-1] = np.where(diff, np.arange(1, Tn)[None, :], Tn)
    re = np.flip(np.minimum.accumulate(np.flip(c, axis=1), axis=1), axis=1)

    s = np.full((Bn, Pn), Tn, np.int64)
    s[:, 0] = 0
    cur = np.zeros(Bn, np.int64)
    arB = np.arange(Bn)
    for i in range(Pn - 1):
        j0 = cur + 1
        active = j0 < Tn
        j0c = np.minimum(j0, Tn - 1)
        eq = (align[arB, j0c] == text[:, i]) & active
        nxt = np.where(active, np.where(eq, re[arB, j0c], j0), Tn)
        s[:, i + 1] = nxt
        cur = nxt
    idx = np.empty((Bn, Tn), np.int32)
    pos = np.arange(Tn)
    for b in range(Bn):
        idx[b] = (np.searchsorted(s[b], pos, side="right") - 1).astype(np.int32)
    return idx


def _positional_encoding_f64(t, e):
    pos = np.arange(t, dtype=np.float64)[:, None]
    div = np.exp(np.arange(0, e, 2, dtype=np.float64) * (-np.log(10000.0) / e))
    ang = pos * div[None, :]
    return np.stack([np.sin(ang), np.cos(ang)], axis=-1).reshape(t, e)


def _bf16_split(x):
    import ml_dtypes
    x = np.asarray(x, np.float32)
    hi = x.astype(ml_dtypes.bfloat16)
    lo = (x - hi.astype(np.float32)).astype(ml_dtypes.bfloat16)
    return hi, lo


# --------------------------------------------------------------------------
# Device programs
# --------------------------------------------------------------------------
def build_nc_fast():
    from contextlib import ExitStack
    import concourse.tile as tile
    from concourse import bacc, mybir
    from concourse._compat import get_trn_type

    f32 = mybir.dt.float32
    bf16 = mybir.dt.bfloat16

    NSB = 4                # supers per store block (16 groups = 2048 tokens)
    SUP = 4                # groups per super-chunk / psum tile

    nc = bacc.Bacc(get_trn_type() or "TRN2", target_bir_lowering=False, debug=False)
    enc_hi = nc.declare_dram_parameter("enc_hi", [RPC, E, P], bf16, isOutput=False)
    g_hi = nc.declare_dram_parameter("g_hi", [E, E], bf16, isOutput=False)
    g_lo = nc.declare_dram_parameter("g_lo", [E, E], bf16, isOutput=False)
    pe_w = nc.declare_dram_parameter("pe_w", [128, NGRP, E], bf16, isOutput=False)
    sj_d = nc.declare_dram_parameter("sj", [128, 2 * 128], bf16, isOutput=False)
    l5_d = nc.declare_dram_parameter("l5", [RPC, 4, T], bf16, isOutput=False)
    w5_d = nc.declare_dram_parameter("w5", [4, E], bf16, isOutput=False)
    out = nc.declare_dram_parameter("out", [RPC, T, E], bf16, isOutput=True)

    with tile.TileContext(nc) as tc, ExitStack() as ctx:
        const = ctx.enter_context(tc.tile_pool(name="const", bufs=1))
        pe_pool = ctx.enter_context(tc.tile_pool(name="pe", bufs=4))
        out_pool = ctx.enter_context(tc.tile_pool(name="outp", bufs=4))
        psum = ctx.enter_context(tc.tile_pool(name="psum", bufs=4, space="PSUM"))

        sj_sb = const.tile([128, 2 * 128], bf16, tag="sj")
        nc.sync.dma_start(sj_sb[:], sj_d[:])
        w5_sb = const.tile([4, E], bf16, tag="w5")
        nc.sync.dma_start(w5_sb[:], w5_d[:])
        gh0 = const.tile([128, E], bf16, tag="gh0")
        gh1 = const.tile([128, E], bf16, tag="gh1")
        gl0 = const.tile([128, E], bf16, tag="gl0")
        gl1 = const.tile([128, E], bf16, tag="gl1")
        nc.sync.dma_start(gh0[:], g_hi[0:128, :])
        nc.sync.dma_start(gh1[:], g_hi[128:256, :])
        nc.sync.dma_start(gl0[:], g_lo[0:128, :])
        nc.sync.dma_start(gl1[:], g_lo[128:256, :])

        l5_sb, egh_keep = [], []
        for r in range(RPC):
            l5t = const.tile([4, T], bf16, tag=f"l5_{r}")
            nc.sync.dma_start(l5t[:], l5_d[r])
            l5_sb.append(l5t)
            egh_keep.append(
                const.tile([128, 8, E], bf16, tag=f"egh{r}", name=f"egh{r}")
            )

        # ---- phase A: encG = enc @ (I+w_pos); enc bf16, g as bf16 hi+lo.
        # psum layout [128p, m, e] == keep layout: row m*128+p at
        # (partition p, block m).  ACT drains psum -> resident bf16.
        with tc.tile_pool(name="encT", bufs=2) as encT_pool:
            for r in range(RPC):
                eh0 = encT_pool.tile([128, P], bf16, tag="eh0")
                eh1 = encT_pool.tile([128, P], bf16, tag="eh1")
                nc.sync.dma_start(eh0[:], enc_hi[r, 0:128, :])
                nc.sync.dma_start(eh1[:], enc_hi[r, 128:256, :])
                for mh in range(2):
                    ps = psum.tile([128, 4 * E], f32, tag="ps")
                    for mi in range(4):
                        m = mh * 4 + mi
                        sl = slice(m * 128, (m + 1) * 128)
                        terms = [
                            (eh0, gh0, True, False), (eh1, gh1, False, False),
                            (eh0, gl0, False, False), (eh1, gl1, False, True),
                        ]
                        for lt, gt_, st, sp in terms:
                            nc.tensor.matmul(
                                ps[:, mi * E:(mi + 1) * E],
                                lhsT=lt[:, sl], rhs=gt_[:], start=st, stop=sp,
                            )
                    nc.scalar.copy(
                        egh_keep[r][:, mh * 4:(mh + 1) * 4, :],
                        ps[:].rearrange("p (n e) -> p n e", e=E),
                    )

        # ---- phase B: per 128-token group g, one K=32 one-hot expansion
        # matmul (rows 16q+u of the 32-row window h select encG rows) + one
        # K=4 stream matmul; psum drained per SUP-group super with the pe
        # tile added in.
        for sb in range(NGRP // (NSB * SUP)):          # 4 store blocks
            pe_ts = []
            for half in range(2):
                pt = pe_pool.tile([128, 8, E], bf16, tag="pe")
                g0 = sb * NSB * SUP + half * 8
                nc.sync.dma_start(pt[:], pe_w[:, g0:g0 + 8, :])
                pe_ts.append(pt)
            for r in range(RPC):
                ot = out_pool.tile([128, NSB * SUP, E], bf16, tag="ot")
                for ss in range(NSB):
                    ps = psum.tile([128, SUP * E], f32, tag="ps")
                    for j in range(SUP):
                        g = (sb * NSB + ss) * SUP + j
                        h = (g % 8) // 2
                        q = g % 2
                        nc.tensor.matmul(
                            ps[:, j * E:(j + 1) * E],
                            lhsT=sj_sb[32 * h:32 * h + 32, q * 128:(q + 1) * 128],
                            rhs=egh_keep[r][32 * h:32 * h + 32, g // 8, :],
                            start=True, stop=False,
                            tile_position=(32 * h, 0),
                        )
                        u0 = g * 128
                        nc.tensor.matmul(
                            ps[:, j * E:(j + 1) * E],
                            lhsT=l5_sb[r][:, u0:u0 + 128],
                            rhs=w5_sb[:],
                            start=False, stop=True,
                        )
                    ot_sl = ot[:, ss * SUP:(ss + 1) * SUP, :]
                    pe_sl = pe_ts[ss // 2][:, (ss % 2) * SUP:(ss % 2 + 1) * SUP, :]
                    ps_v = ps[:].rearrange("p (n e) -> p n e", e=E)
                    if r == 0:
                        # fused drain: psum f32 + pe bf16 -> bf16 (DVE 1x)
                        nc.vector.tensor_add(ot_sl, ps_v, pe_sl)
                    else:
                        # ACT copy drain, then cheap all-bf16 DVE 2x add
                        nc.scalar.copy(ot_sl, ps_v)
                        nc.vector.tensor_add(ot_sl, ot_sl, pe_sl)
                t0 = sb * NSB * SUP * 128
                nc.scalar.dma_start(
                    out[r, t0:t0 + NSB * SUP * 128, :].rearrange(
                        "(n p) e -> p n e", p=128
                    ),
                    ot[:],
                )
    nc.compile()
    return nc


def build_nc_general():
    """Arbitrary-idx path: per-128-token indirect row gathers."""
    import concourse.bass as bass
    from contextlib import ExitStack
    import concourse.tile as tile
    from concourse import bacc, mybir
    from concourse._compat import get_trn_type

    f32 = mybir.dt.float32
    i32 = mybir.dt.int32

    nc = bacc.Bacc(get_trn_type() or "TRN2", target_bir_lowering=False, debug=False)
    enc_t = nc.declare_dram_parameter("enc_t", [RPC, E, P], f32, isOutput=False)
    g_mat = nc.declare_dram_parameter("g_mat", [E, E], f32, isOutput=False)
    pe_w = nc.declare_dram_parameter("pe_w", [128, NGRP, E], f32, isOutput=False)
    p3 = nc.declare_dram_parameter("p3", [RPC, 3, T], f32, isOutput=False)
    w3 = nc.declare_dram_parameter("w3", [3, E], f32, isOutput=False)
    idxo = nc.declare_dram_parameter(
        "idxo", [RPC, 128, NGRP], i32, isOutput=False
    )
    out = nc.declare_dram_parameter("out", [RPC, T, E], f32, isOutput=True)
    encg = nc.dram_tensor("encg", [RPC, P, E], f32)

    with tile.TileContext(nc) as tc, ExitStack() as ctx:
        const = ctx.enter_context(tc.tile_pool(name="const", bufs=1))
        encT_pool = ctx.enter_context(tc.tile_pool(name="encT", bufs=2))
        psum_pool = ctx.enter_context(tc.tile_pool(name="psum", bufs=2, space="PSUM"))
        eg_pool = ctx.enter_context(tc.tile_pool(name="eg", bufs=2))
        pe_pool = ctx.enter_context(tc.tile_pool(name="pe", bufs=2))
        gath_pool = ctx.enter_context(tc.tile_pool(name="gath", bufs=3))

        g0 = const.tile([128, E], f32, tag="g0")
        g1 = const.tile([128, E], f32, tag="g1")
        nc.sync.dma_start(g0[:], g_mat[0:128, :])
        nc.sync.dma_start(g1[:], g_mat[128:256, :])
        w3_sb = const.tile([3, E], f32, tag="w3")
        nc.sync.dma_start(w3_sb[:], w3[:, :])
        p3_sb = []
        ixo_sb = []
        for r in range(RPC):
            p3t = const.tile([3, T], f32, tag=f"p3_{r}")
            nc.sync.dma_start(p3t[:], p3[r])
            p3_sb.append(p3t)
            ixt = const.tile([128, NGRP], i32, tag=f"ixo_{r}")
            nc.sync.dma_start(ixt[:], idxo[r])
            ixo_sb.append(ixt)

        for r in range(RPC):
            et0 = encT_pool.tile([128, P], f32, tag="et0")
            et1 = encT_pool.tile([128, P], f32, tag="et1")
            nc.sync.dma_start(et0[:], enc_t[r, 0:128, :])
            nc.sync.dma_start(et1[:], enc_t[r, 128:256, :])
            ps = psum_pool.tile([128, 8 * E], f32, tag="ps")
            for m in range(8):
                nc.tensor.matmul(
                    ps[:, m * E:(m + 1) * E],
                    lhsT=et0[:, m * 128:(m + 1) * 128],
                    rhs=g0[:], start=True, stop=False,
                )
                nc.tensor.matmul(
                    ps[:, m * E:(m + 1) * E],
                    lhsT=et1[:, m * 128:(m + 1) * 128],
                    rhs=g1[:], start=False, stop=True,
                )
            eg = eg_pool.tile([128, 8 * E], f32, tag="eg")
            nc.vector.tensor_copy(eg[:], ps[:])
            nc.sync.dma_start(
                encg[r].rearrange("(m p) e -> p m e", p=128),
                eg[:].rearrange("q (m e) -> q m e", e=E),
            )

        encg_flat = encg[:].rearrange("r p e -> (r p) e")
        for s in range(T // (NSUP * 128)):
            pe_t = pe_pool.tile([128, NSUP, E], f32, tag="pe")
            nc.sync.dma_start(pe_t[:], pe_w[:, s * NSUP:(s + 1) * NSUP, :])
            for r in range(RPC):
                gt = gath_pool.tile([128, NSUP, E], f32, tag="gt")
                for g in range(NSUP):
                    gi = s * NSUP + g
                    nc.gpsimd.indirect_dma_start(
                        out=gt[:, g, :],
                        out_offset=None,
                        in_=encg_flat,
                        in_offset=bass.IndirectOffsetOnAxis(
                            ap=ixo_sb[r][:, gi:gi + 1], axis=0
                        ),
                    )
                nc.vector.tensor_add(gt[:], gt[:], pe_t[:])
                ps = psum_pool.tile([128, 8 * E], f32, tag="ps")
                for g in range(NSUP):
                    gi = s * NSUP + g
                    nc.tensor.matmul(
                        ps[:, g * E:(g + 1) * E],
                        lhsT=p3_sb[r][:, gi * 128:(gi + 1) * 128],
                        rhs=w3_sb[:],
                        start=True, stop=True,
                    )
                nc.vector.tensor_add(
                    gt[:], gt[:], ps[:].rearrange("q (n e) -> q n e", e=E)
                )
                nc.sync.dma_start(
                    out[r, s * NSUP * 128:(s + 1) * NSUP * 128, :].rearrange(
                        "(n p) e -> p n e", p=128
                    ),
                    gt[:],
                )
    nc.compile()
    return nc


def get_nc(fast):
    key = "nc_fast" if fast else "nc_gen"
    if key not in _CACHE:
        _CACHE[key] = build_nc_fast() if fast else build_nc_general()
    return _CACHE[key]


# --------------------------------------------------------------------------
# Host wrapper
# --------------------------------------------------------------------------
def make_in_maps(encoder_out, align_phone, text_phone, pitch, beats,
                 w_pitch, b_pitch, emb_beats, w_pos, b_pos):
    import ml_dtypes

    encoder_out = np.asarray(encoder_out, np.float32)
    pitch = np.asarray(pitch, np.float32)
    beats = np.asarray(beats)
    w_pitch = np.asarray(w_pitch, np.float32)
    w_pos = np.asarray(w_pos, np.float32)

    idx = compute_idx(np.asarray(align_phone), np.asarray(text_phone))
    fast = bool(np.all(idx == (np.arange(T, dtype=np.int32) // DUR)[None, :]))
    if FORCE_GENERAL:
        fast = False

    g_mat = (np.eye(E, dtype=np.float64) + w_pos.astype(np.float64)).astype(np.float32)
    pe = _positional_encoding_f64(T, E)
    pe_proj = pe @ w_pos.astype(np.float64)                          # [T, E]
    bias = (np.asarray(emb_beats[0], np.float64)
            + np.asarray(b_pitch, np.float64)
            + np.asarray(b_pos, np.float64))
    demb = (np.asarray(emb_beats[1], np.float64)
            - np.asarray(emb_beats[0], np.float64)).astype(np.float32)

    if fast:
        pe_tot = (pe_proj + bias[None, :]).astype(ml_dtypes.bfloat16)
        pe_wrap = np.ascontiguousarray(pe_tot.reshape(NGRP, 128, E).swapaxes(0, 1))
        # sj[p, q*128+dt] = 1 iff (p%32) == 16q + dt//8   (K=32 windows)
        rows = np.arange(128)[:, None] % 32
        sj = np.concatenate(
            [(rows == (q * NW + np.arange(128) // DUR)[None, :]) for q in range(2)],
            axis=1,
        ).astype(ml_dtypes.bfloat16)
        g_hi, g_lo = _bf16_split(g_mat)
        wp_hi, wp_lo = _bf16_split(w_pitch[0])
        db_hi, db_lo = _bf16_split(demb)
        w5 = np.stack([wp_hi, wp_lo, db_hi, db_lo]).astype(ml_dtypes.bfloat16)
        fast_common = {
            "pe_w": pe_wrap, "sj": sj,
            "g_hi": g_hi, "g_lo": g_lo, "w5": w5,
        }
    else:
        w3 = np.stack(
            [w_pitch[0].astype(np.float64), demb.astype(np.float64), bias]
        ).astype(np.float32)
        pe_wl = np.ascontiguousarray(
            pe_proj.astype(np.float32).reshape(NGRP, 128, E).swapaxes(0, 1)
        )

    in_maps = []
    for core in range(NCORES):
        rows_ = range(core * RPC, (core + 1) * RPC)
        enc_t = np.ascontiguousarray(
            encoder_out[core * RPC:(core + 1) * RPC].transpose(0, 2, 1)
        )
        if fast:
            import ml_dtypes as _md
            enc_hi = enc_t.astype(_md.bfloat16)
            l5 = np.zeros((RPC, 4, T), _md.bfloat16)
            for j, b in enumerate(rows_):
                p_hi = pitch[b, :, 0].astype(_md.bfloat16)
                bt = beats[b, :, 0].astype(_md.bfloat16)
                l5[j] = np.stack([p_hi, p_hi, bt, bt])
            m = {"enc_hi": enc_hi, "l5": l5, **fast_common}
        else:
            p3 = np.empty((RPC, 3, T), np.float32)
            idxo = np.empty((RPC, 128, NGRP), np.int32)
            for j, b in enumerate(rows_):
                p3[j, 0] = pitch[b, :, 0]
                p3[j, 1] = beats[b, :, 0].astype(np.float32)
                p3[j, 2] = 1.0
                idxo[j] = idx[b].reshape(NGRP, 128).T + j * P
            m = {"enc_t": enc_t, "g_mat": g_mat, "pe_w": pe_wl, "p3": p3,
                 "w3": w3, "idxo": idxo}
        in_maps.append(m)
    return fast, in_maps


def kernel(**inputs):
    from concourse.bass_utils import run_bass_kernel_spmd

    fast, in_maps = make_in_maps(**inputs)
    nc = get_nc(fast)
    res = run_bass_kernel_spmd(nc, in_maps, core_ids=list(range(NCORES)))
    out = np.concatenate([res.results[i]["out"] for i in range(NCORES)], axis=0)
    return np.ascontiguousarray(out.astype(np.float32))


# revision 8
# speedup vs baseline: 1.5359x; 1.4490x over previous
"""Trainium2 Bass kernel for nn_Encoder_Postnet (ragged_sequence).

Computation (reference):
    idx   = sequential aligner scan over (align_phone, text_phone)   [B,T]
    out   = enc[idx] + pitch @ w_pitch + b_pitch + emb_beats[beats]
            + (enc[idx] + pe) @ w_pos + b_pos

Key algebraic restructure: the duration-expansion gather commutes with the
E x E linear, so
    out[t] = encG[idx_t] + (pe@w_pos + bias)[t] + pitch[t]*w_pitch + beats[t]*demb
with encG = enc @ (I + w_pos) computed once per batch row ([P,E] not [T,E]),
collapsing the big [B*T,E]@[E,E] matmul 8x and making the kernel memory-bound.

Sharding: pure data parallel, 2 batch rows per core across 8 cores.

Fast path (the uniform duration-8 expansion this model's inputs produce,
idx == arange(T)//8 for every row).  The whole dataflow is bf16 (the harness
tolerance is 2e-2 relmax; bf16 roundoff is ~4e-3), which halves every DMA
stream vs f32 -- the cost model serializes all DMA traffic at 360 GB/s so
bytes moved is the wall clock:
  phase A: encG = enc_bf16 @ (I+w_pos) on PE (g kept as bf16 hi+lo for
           accuracy); psum drained by the ACT engine to a resident bf16 tile.
  phase B: per 128-token group, one K=32 one-hot expansion matmul (selects
           and repeats the 16 encG rows for this group) + one K=4 stream
           matmul (pitch x w_pitch hi/lo + beats x demb hi/lo) into PSUM.
           Per 4-group super-chunk the psum is drained with the
           (pe@w_pos + bias) bf16 tile added in:
             row 0: DVE fused tensor_add (psum + pe -> bf16 out tile)
             row 1: ACT copy (psum -> bf16) then DVE 2x-mode bf16 add of pe
           which balances DVE/ACT busy time under the DMA roofline.
  Output is stored as bf16 (converted to f32 on host).

General path (arbitrary idx): per-128-token indirect-DMA row gathers
(production-shaped offset [128,1] DynamicAP descriptors) + K=3 stream matmul.

The aligner scan itself is index metadata ([B,T] int32); it is computed on
host with a run-compressed O(B*P) algorithm exactly equivalent to the
reference recurrence, then consumed either as a uniformity proof (fast path)
or as gather offsets (general path).
"""

import sys

for _p in ("/opt/trn_rl_repo",):
    if _p not in sys.path:
        sys.path.insert(0, _p)

import numpy as np

B, P, T, E = 16, 1024, 8192, 256
NCORES = 8
RPC = B // NCORES          # batch rows per core
NGRP = T // 128            # 64 groups of 128 tokens per row
NSUP = 8                   # groups per super-chunk (general path)
DUR = T // P               # uniform duration of the fast path (8)
NW = 128 // DUR            # encG rows per group (16)

FORCE_GENERAL = False      # test hook: force the arbitrary-idx path
_CACHE = {}


# --------------------------------------------------------------------------
# Host: aligner index computation (exact replica of the reference recurrence)
# --------------------------------------------------------------------------
def compute_idx(align, text):
    """idx[b,0]=0; idx[b,j] = idx[b,j-1] if align[b,j]==text[b,idx[b,j-1]]
    else min(idx[b,j-1]+1, P-1).   Vectorized over batch via segment starts:
    the pointer advances i->i+1 at s_{i+1} = first j >= s_i+1 with
    align[j] != text[i]; within a run of align values equal to text[i] the
    first mismatch is the run end."""
    align = np.asarray(align)
    text = np.asarray(text)
    Bn, Tn = align.shape
    Pn = text.shape[1]
    diff = align[:, 1:] != align[:, :-1]                       # [B, T-1]
    c = np.full((Bn, Tn), Tn, np.int64)
    c[:, :-1] = np.where(diff, np.arange(1, Tn)[None, :], Tn)
    re = np.flip(np.minimum.accumulate(np.flip(c, axis=1), axis=1), axis=1)

    s = np.full((Bn, Pn), Tn, np.int64)
    s[:, 0] = 0
    cur = np.zeros(Bn, np.int64)
    arB = np.arange(Bn)
    for i in range(Pn - 1):
        j0 = cur + 1
        active = j0 < Tn
        j0c = np.minimum(j0, Tn - 1)
        eq = (align[arB, j0c] == text[:, i]) & active
        nxt = np.where(active, np.where(eq, re[arB, j0c], j0), Tn)
        s[:, i + 1] = nxt
        cur = nxt
    idx = np.empty((Bn, Tn), np.int32)
    pos = np.arange(Tn)
    for b in range(Bn):
        idx[b] = (np.searchsorted(s[b], pos, side="right") - 1).astype(np.int32)
    return idx


def _positional_encoding_f64(t, e):
    pos = np.arange(t, dtype=np.float64)[:, None]
    div = np.exp(np.arange(0, e, 2, dtype=np.float64) * (-np.log(10000.0) / e))
    ang = pos * div[None, :]
    return np.stack([np.sin(ang), np.cos(ang)], axis=-1).reshape(t, e)


def _bf16_split(x):
    import ml_dtypes
    x = np.asarray(x, np.float32)
    hi = x.astype(ml_dtypes.bfloat16)
    lo = (x - hi.astype(np.float32)).astype(ml_dtypes.bfloat16)
    return hi, lo


# --------------------------------------------------------------------------
# Device programs
# --------------------------------------------------------------------------
def build_nc_fast():
    from contextlib import ExitStack
    import concourse.tile as tile
    from concourse import bacc, mybir
    from concourse._compat import get_trn_type

    f32 = mybir.dt.float32
    bf16 = mybir.dt.bfloat16

    NSB = 4                # supers per store block (16 groups = 2048 tokens)
    SUP = 4                # groups per super-chunk / psum tile

    nc = bacc.Bacc(get_trn_type() or "TRN2", target_bir_lowering=False, debug=False)
    enc_hi = nc.declare_dram_parameter("enc_hi", [RPC, E, P], bf16, isOutput=False)
    g_hi = nc.declare_dram_parameter("g_hi", [E, E], bf16, isOutput=False)
    pe_w = nc.declare_dram_parameter("pe_w", [128, NGRP, E], bf16, isOutput=False)
    sj_d = nc.declare_dram_parameter("sj", [128, 2 * 128], bf16, isOutput=False)
    l5_d = nc.declare_dram_parameter("l5", [RPC, 4, T], bf16, isOutput=False)
    w5_d = nc.declare_dram_parameter("w5", [4, E], bf16, isOutput=False)
    out = nc.declare_dram_parameter("out", [RPC, T, E], bf16, isOutput=True)

    with tile.TileContext(nc) as tc, ExitStack() as ctx:
        const = ctx.enter_context(tc.tile_pool(name="const", bufs=1))
        pe_pool = ctx.enter_context(tc.tile_pool(name="pe", bufs=4))
        out_pool = ctx.enter_context(tc.tile_pool(name="outp", bufs=4))
        psum = ctx.enter_context(tc.tile_pool(name="psum", bufs=4, space="PSUM"))

        sj_sb = const.tile([128, 2 * 128], bf16, tag="sj")
        nc.sync.dma_start(sj_sb[:], sj_d[:])
        w5_sb = const.tile([4, E], bf16, tag="w5")
        nc.sync.dma_start(w5_sb[:], w5_d[:])
        gh0 = const.tile([128, E], bf16, tag="gh0")
        gh1 = const.tile([128, E], bf16, tag="gh1")
        nc.sync.dma_start(gh0[:], g_hi[0:128, :])
        nc.sync.dma_start(gh1[:], g_hi[128:256, :])

        # l5 loads are expensive in the per-partition-bytes DMA model (16KB
        # free dim on 4 partitions); put them on different queues.
        l5_sb, egh_keep = [], []
        for r in range(RPC):
            l5t = const.tile([4, T], bf16, tag=f"l5_{r}")
            (nc.sync if r == 0 else nc.scalar).dma_start(l5t[:], l5_d[r])
            l5_sb.append(l5t)
            egh_keep.append(
                const.tile([128, 8, E], bf16, tag=f"egh{r}", name=f"egh{r}")
            )

        # ---- phase A: encG = enc @ (I+w_pos); enc bf16, g as bf16 hi+lo.
        # psum layout [128p, m, e] == keep layout: row m*128+p at
        # (partition p, block m).  ACT drains psum -> resident bf16.
        with tc.tile_pool(name="encT", bufs=2) as encT_pool:
            for r in range(RPC):
                eh0 = encT_pool.tile([128, P], bf16, tag="eh0")
                eh1 = encT_pool.tile([128, P], bf16, tag="eh1")
                nc.sync.dma_start(eh0[:], enc_hi[r, 0:128, :])
                nc.sync.dma_start(eh1[:], enc_hi[r, 128:256, :])
                for mh in range(2):
                    ps = psum.tile([128, 4 * E], f32, tag="ps")
                    for mi in range(4):
                        m = mh * 4 + mi
                        sl = slice(m * 128, (m + 1) * 128)
                        terms = [
                            (eh0, gh0, True, False), (eh1, gh1, False, True),
                        ]
                        for lt, gt_, st, sp in terms:
                            nc.tensor.matmul(
                                ps[:, mi * E:(mi + 1) * E],
                                lhsT=lt[:, sl], rhs=gt_[:], start=st, stop=sp,
                            )
                    nc.scalar.copy(
                        egh_keep[r][:, mh * 4:(mh + 1) * 4, :],
                        ps[:].rearrange("p (n e) -> p n e", e=E),
                    )

        # ---- phase B: per 128-token group g, one K=32 one-hot expansion
        # matmul (rows 16q+u of the 32-row window h select encG rows) + one
        # K=4 stream matmul; psum drained per SUP-group super with the pe
        # tile added in.
        for sb in range(NGRP // (NSB * SUP)):          # 4 store blocks
            pe_ts = []
            for half in range(2):
                pt = pe_pool.tile([128, 8, E], bf16, tag="pe")
                g0 = sb * NSB * SUP + half * 8
                nc.sync.dma_start(pt[:], pe_w[:, g0:g0 + 8, :])
                pe_ts.append(pt)
            for r in range(RPC):
                ot = out_pool.tile([128, NSB * SUP, E], bf16, tag="ot")
                for ss in range(NSB):
                    ps = psum.tile([128, SUP * E], f32, tag="ps")
                    for j in range(SUP):
                        g = (sb * NSB + ss) * SUP + j
                        h = (g % 8) // 2
                        q = g % 2
                        nc.tensor.matmul(
                            ps[:, j * E:(j + 1) * E],
                            lhsT=sj_sb[32 * h:32 * h + 32, q * 128:(q + 1) * 128],
                            rhs=egh_keep[r][32 * h:32 * h + 32, g // 8, :],
                            start=True, stop=False,
                            tile_position=(32 * h, 0),
                        )
                        u0 = g * 128
                        nc.tensor.matmul(
                            ps[:, j * E:(j + 1) * E],
                            lhsT=l5_sb[r][:, u0:u0 + 128],
                            rhs=w5_sb[:],
                            start=False, stop=True,
                        )
                    ot_sl = ot[:, ss * SUP:(ss + 1) * SUP, :]
                    pe_sl = pe_ts[ss // 2][:, (ss % 2) * SUP:(ss % 2 + 1) * SUP, :]
                    ps_v = ps[:].rearrange("p (n e) -> p n e", e=E)
                    if r == 0:
                        # fused drain: psum f32 + pe bf16 -> bf16 (DVE 1x)
                        nc.vector.tensor_add(ot_sl, ps_v, pe_sl)
                    else:
                        # ACT copy drain, then cheap all-bf16 DVE 2x add
                        nc.scalar.copy(ot_sl, ps_v)
                        nc.vector.tensor_add(ot_sl, ot_sl, pe_sl)
                t0 = sb * NSB * SUP * 128
                # stores ride the otherwise-idle Pool (gpsimd) queue
                nc.gpsimd.dma_start(
                    out[r, t0:t0 + NSB * SUP * 128, :].rearrange(
                        "(n p) e -> p n e", p=128
                    ),
                    ot[:],
                )
    nc.compile()
    return nc


def build_nc_general():
    """Arbitrary-idx path: per-128-token indirect row gathers."""
    import concourse.bass as bass
    from contextlib import ExitStack
    import concourse.tile as tile
    from concourse import bacc, mybir
    from concourse._compat import get_trn_type

    f32 = mybir.dt.float32
    i32 = mybir.dt.int32

    nc = bacc.Bacc(get_trn_type() or "TRN2", target_bir_lowering=False, debug=False)
    enc_t = nc.declare_dram_parameter("enc_t", [RPC, E, P], f32, isOutput=False)
    g_mat = nc.declare_dram_parameter("g_mat", [E, E], f32, isOutput=False)
    pe_w = nc.declare_dram_parameter("pe_w", [128, NGRP, E], f32, isOutput=False)
    p3 = nc.declare_dram_parameter("p3", [RPC, 3, T], f32, isOutput=False)
    w3 = nc.declare_dram_parameter("w3", [3, E], f32, isOutput=False)
    idxo = nc.declare_dram_parameter(
        "idxo", [RPC, 128, NGRP], i32, isOutput=False
    )
    out = nc.declare_dram_parameter("out", [RPC, T, E], f32, isOutput=True)
    encg = nc.dram_tensor("encg", [RPC, P, E], f32)

    with tile.TileContext(nc) as tc, ExitStack() as ctx:
        const = ctx.enter_context(tc.tile_pool(name="const", bufs=1))
        encT_pool = ctx.enter_context(tc.tile_pool(name="encT", bufs=2))
        psum_pool = ctx.enter_context(tc.tile_pool(name="psum", bufs=2, space="PSUM"))
        eg_pool = ctx.enter_context(tc.tile_pool(name="eg", bufs=2))
        pe_pool = ctx.enter_context(tc.tile_pool(name="pe", bufs=2))
        gath_pool = ctx.enter_context(tc.tile_pool(name="gath", bufs=3))

        g0 = const.tile([128, E], f32, tag="g0")
        g1 = const.tile([128, E], f32, tag="g1")
        nc.sync.dma_start(g0[:], g_mat[0:128, :])
        nc.sync.dma_start(g1[:], g_mat[128:256, :])
        w3_sb = const.tile([3, E], f32, tag="w3")
        nc.sync.dma_start(w3_sb[:], w3[:, :])
        p3_sb = []
        ixo_sb = []
        for r in range(RPC):
            p3t = const.tile([3, T], f32, tag=f"p3_{r}")
            nc.sync.dma_start(p3t[:], p3[r])
            p3_sb.append(p3t)
            ixt = const.tile([128, NGRP], i32, tag=f"ixo_{r}")
            nc.sync.dma_start(ixt[:], idxo[r])
            ixo_sb.append(ixt)

        for r in range(RPC):
            et0 = encT_pool.tile([128, P], f32, tag="et0")
            et1 = encT_pool.tile([128, P], f32, tag="et1")
            nc.sync.dma_start(et0[:], enc_t[r, 0:128, :])
            nc.sync.dma_start(et1[:], enc_t[r, 128:256, :])
            ps = psum_pool.tile([128, 8 * E], f32, tag="ps")
            for m in range(8):
                nc.tensor.matmul(
                    ps[:, m * E:(m + 1) * E],
                    lhsT=et0[:, m * 128:(m + 1) * 128],
                    rhs=g0[:], start=True, stop=False,
                )
                nc.tensor.matmul(
                    ps[:, m * E:(m + 1) * E],
                    lhsT=et1[:, m * 128:(m + 1) * 128],
                    rhs=g1[:], start=False, stop=True,
                )
            eg = eg_pool.tile([128, 8 * E], f32, tag="eg")
            nc.vector.tensor_copy(eg[:], ps[:])
            nc.sync.dma_start(
                encg[r].rearrange("(m p) e -> p m e", p=128),
                eg[:].rearrange("q (m e) -> q m e", e=E),
            )

        encg_flat = encg[:].rearrange("r p e -> (r p) e")
        for s in range(T // (NSUP * 128)):
            pe_t = pe_pool.tile([128, NSUP, E], f32, tag="pe")
            nc.sync.dma_start(pe_t[:], pe_w[:, s * NSUP:(s + 1) * NSUP, :])
            for r in range(RPC):
                gt = gath_pool.tile([128, NSUP, E], f32, tag="gt")
                for g in range(NSUP):
                    gi = s * NSUP + g
                    nc.gpsimd.indirect_dma_start(
                        out=gt[:, g, :],
                        out_offset=None,
                        in_=encg_flat,
                        in_offset=bass.IndirectOffsetOnAxis(
                            ap=ixo_sb[r][:, gi:gi + 1], axis=0
                        ),
                    )
                nc.vector.tensor_add(gt[:], gt[:], pe_t[:])
                ps = psum_pool.tile([128, 8 * E], f32, tag="ps")
                for g in range(NSUP):
                    gi = s * NSUP + g
                    nc.tensor.matmul(
                        ps[:, g * E:(g + 1) * E],
                        lhsT=p3_sb[r][:, gi * 128:(gi + 1) * 128],
                        rhs=w3_sb[:],
                        start=True, stop=True,
                    )
                nc.vector.tensor_add(
                    gt[:], gt[:], ps[:].rearrange("q (n e) -> q n e", e=E)
                )
                nc.sync.dma_start(
                    out[r, s * NSUP * 128:(s + 1) * NSUP * 128, :].rearrange(
                        "(n p) e -> p n e", p=128
                    ),
                    gt[:],
                )
    nc.compile()
    return nc


def get_nc(fast):
    key = "nc_fast" if fast else "nc_gen"
    if key not in _CACHE:
        _CACHE[key] = build_nc_fast() if fast else build_nc_general()
    return _CACHE[key]


# --------------------------------------------------------------------------
# Host wrapper
# --------------------------------------------------------------------------
def make_in_maps(encoder_out, align_phone, text_phone, pitch, beats,
                 w_pitch, b_pitch, emb_beats, w_pos, b_pos):
    import ml_dtypes

    encoder_out = np.asarray(encoder_out, np.float32)
    pitch = np.asarray(pitch, np.float32)
    beats = np.asarray(beats)
    w_pitch = np.asarray(w_pitch, np.float32)
    w_pos = np.asarray(w_pos, np.float32)

    idx = compute_idx(np.asarray(align_phone), np.asarray(text_phone))
    fast = bool(np.all(idx == (np.arange(T, dtype=np.int32) // DUR)[None, :]))
    if FORCE_GENERAL:
        fast = False

    g_mat = (np.eye(E, dtype=np.float64) + w_pos.astype(np.float64)).astype(np.float32)
    pe = _positional_encoding_f64(T, E)
    pe_proj = pe @ w_pos.astype(np.float64)                          # [T, E]
    bias = (np.asarray(emb_beats[0], np.float64)
            + np.asarray(b_pitch, np.float64)
            + np.asarray(b_pos, np.float64))
    demb = (np.asarray(emb_beats[1], np.float64)
            - np.asarray(emb_beats[0], np.float64)).astype(np.float32)

    if fast:
        pe_tot = (pe_proj + bias[None, :]).astype(ml_dtypes.bfloat16)
        pe_wrap = np.ascontiguousarray(pe_tot.reshape(NGRP, 128, E).swapaxes(0, 1))
        # sj[p, q*128+dt] = 1 iff (p%32) == 16q + dt//8   (K=32 windows)
        rows = np.arange(128)[:, None] % 32
        sj = np.concatenate(
            [(rows == (q * NW + np.arange(128) // DUR)[None, :]) for q in range(2)],
            axis=1,
        ).astype(ml_dtypes.bfloat16)
        g_hi = g_mat.astype(ml_dtypes.bfloat16)
        wp_hi, wp_lo = _bf16_split(w_pitch[0])
        db_hi, db_lo = _bf16_split(demb)
        w5 = np.stack([wp_hi, wp_lo, db_hi, db_lo]).astype(ml_dtypes.bfloat16)
        fast_common = {
            "pe_w": pe_wrap, "sj": sj,
            "g_hi": g_hi, "w5": w5,
        }
    else:
        w3 = np.stack(
            [w_pitch[0].astype(np.float64), demb.astype(np.float64), bias]
        ).astype(np.float32)
        pe_wl = np.ascontiguousarray(
            pe_proj.astype(np.float32).reshape(NGRP, 128, E).swapaxes(0, 1)
        )

    in_maps = []
    for core in range(NCORES):
        rows_ = range(core * RPC, (core + 1) * RPC)
        enc_t = np.ascontiguousarray(
            encoder_out[core * RPC:(core + 1) * RPC].transpose(0, 2, 1)
        )
        if fast:
            import ml_dtypes as _md
            enc_hi = enc_t.astype(_md.bfloat16)
            l5 = np.zeros((RPC, 4, T), _md.bfloat16)
            for j, b in enumerate(rows_):
                p_hi = pitch[b, :, 0].astype(_md.bfloat16)
                bt = beats[b, :, 0].astype(_md.bfloat16)
                l5[j] = np.stack([p_hi, p_hi, bt, bt])
            m = {"enc_hi": enc_hi, "l5": l5, **fast_common}
        else:
            p3 = np.empty((RPC, 3, T), np.float32)
            idxo = np.empty((RPC, 128, NGRP), np.int32)
            for j, b in enumerate(rows_):
                p3[j, 0] = pitch[b, :, 0]
                p3[j, 1] = beats[b, :, 0].astype(np.float32)
                p3[j, 2] = 1.0
                idxo[j] = idx[b].reshape(NGRP, 128).T + j * P
            m = {"enc_t": enc_t, "g_mat": g_mat, "pe_w": pe_wl, "p3": p3,
                 "w3": w3, "idxo": idxo}
        in_maps.append(m)
    return fast, in_maps


def kernel(**inputs):
    from concourse.bass_utils import run_bass_kernel_spmd

    fast, in_maps = make_in_maps(**inputs)
    nc = get_nc(fast)
    res = run_bass_kernel_spmd(nc, in_maps, core_ids=list(range(NCORES)))
    out = np.concatenate([res.results[i]["out"] for i in range(NCORES)], axis=0)
    return np.ascontiguousarray(out.astype(np.float32))


# revision 13
# speedup vs baseline: 1.8395x; 1.1977x over previous
"""Trainium2 Bass kernel for nn_Encoder_Postnet (ragged_sequence).

Computation (reference):
    idx   = sequential aligner scan over (align_phone, text_phone)   [B,T]
    out   = enc[idx] + pitch @ w_pitch + b_pitch + emb_beats[beats]
            + (enc[idx] + pe) @ w_pos + b_pos

Key algebraic restructure: the duration-expansion gather commutes with the
E x E linear, so
    out[t] = encG[idx_t] + (pe@w_pos + bias)[t] + pitch[t]*w_pitch + beats[t]*demb
with encG = enc @ (I + w_pos) computed once per batch row ([P,E] not [T,E]),
collapsing the big [B*T,E]@[E,E] matmul 8x and making the kernel memory-bound.

Sharding: pure data parallel, 2 batch rows per core across 8 cores.

Fast path (the uniform duration-8 expansion this model's inputs produce,
idx == arange(T)//8 for every row).  The whole dataflow is bf16 (the harness
tolerance is 2e-2 relmax; bf16 roundoff is ~4e-3), which halves every DMA
stream vs f32 -- the cost model serializes all DMA traffic at 360 GB/s so
bytes moved is the wall clock:
  phase A: encG = enc_bf16 @ (I+w_pos) on PE (g kept as bf16 hi+lo for
           accuracy); psum drained by the ACT engine to a resident bf16 tile.
  phase B: per 128-token group, one K=32 one-hot expansion matmul (selects
           and repeats the 16 encG rows for this group) + one K=4 stream
           matmul (pitch x w_pitch hi/lo + beats x demb hi/lo) into PSUM.
           Per 4-group super-chunk the psum is drained with the
           (pe@w_pos + bias) bf16 tile added in:
             row 0: DVE fused tensor_add (psum + pe -> bf16 out tile)
             row 1: ACT copy (psum -> bf16) then DVE 2x-mode bf16 add of pe
           which balances DVE/ACT busy time under the DMA roofline.
  Output is stored as bf16 (converted to f32 on host).

General path (arbitrary idx): per-128-token indirect-DMA row gathers
(production-shaped offset [128,1] DynamicAP descriptors) + K=3 stream matmul.

The aligner scan itself is index metadata ([B,T] int32); it is computed on
host with a run-compressed O(B*P) algorithm exactly equivalent to the
reference recurrence, then consumed either as a uniformity proof (fast path)
or as gather offsets (general path).
"""

import sys

for _p in ("/opt/trn_rl_repo",):
    if _p not in sys.path:
        sys.path.insert(0, _p)

import numpy as np

B, P, T, E = 16, 1024, 8192, 256
NCORES = 8
RPC = B // NCORES          # batch rows per core
NGRP = T // 128            # 64 groups of 128 tokens per row
NSUP = 8                   # groups per super-chunk (general path)
DUR = T // P               # uniform duration of the fast path (8)
NW = 128 // DUR            # encG rows per group (16)

FORCE_GENERAL = False      # test hook: force the arbitrary-idx path
_CACHE = {}


# --------------------------------------------------------------------------
# Host: aligner index computation (exact replica of the reference recurrence)
# --------------------------------------------------------------------------
def compute_idx(align, text):
    """idx[b,0]=0; idx[b,j] = idx[b,j-1] if align[b,j]==text[b,idx[b,j-1]]
    else min(idx[b,j-1]+1, P-1).   Vectorized over batch via segment starts:
    the pointer advances i->i+1 at s_{i+1} = first j >= s_i+1 with
    align[j] != text[i]; within a run of align values equal to text[i] the
    first mismatch is the run end."""
    align = np.asarray(align)
    text = np.asarray(text)
    Bn, Tn = align.shape
    Pn = text.shape[1]
    diff = align[:, 1:] != align[:, :-1]                       # [B, T-1]
    c = np.full((Bn, Tn), Tn, np.int64)
    c[:, :-1] = np.where(diff, np.arange(1, Tn)[None, :], Tn)
    re = np.flip(np.minimum.accumulate(np.flip(c, axis=1), axis=1), axis=1)

    s = np.full((Bn, Pn), Tn, np.int64)
    s[:, 0] = 0
    cur = np.zeros(Bn, np.int64)
    arB = np.arange(Bn)
    for i in range(Pn - 1):
        j0 = cur + 1
        active = j0 < Tn
        j0c = np.minimum(j0, Tn - 1)
        eq = (align[arB, j0c] == text[:, i]) & active
        nxt = np.where(active, np.where(eq, re[arB, j0c], j0), Tn)
        s[:, i + 1] = nxt
        cur = nxt
    idx = np.empty((Bn, Tn), np.int32)
    pos = np.arange(Tn)
    for b in range(Bn):
        idx[b] = (np.searchsorted(s[b], pos, side="right") - 1).astype(np.int32)
    return idx


def _positional_encoding_f64(t, e):
    pos = np.arange(t, dtype=np.float64)[:, None]
    div = np.exp(np.arange(0, e, 2, dtype=np.float64) * (-np.log(10000.0) / e))
    ang = pos * div[None, :]
    return np.stack([np.sin(ang), np.cos(ang)], axis=-1).reshape(t, e)


def _bf16_split(x):
    import ml_dtypes
    x = np.asarray(x, np.float32)
    hi = x.astype(ml_dtypes.bfloat16)
    lo = (x - hi.astype(np.float32)).astype(ml_dtypes.bfloat16)
    return hi, lo


# --------------------------------------------------------------------------
# Device programs
# --------------------------------------------------------------------------
def build_nc_fast():
    from contextlib import ExitStack
    import concourse.tile as tile
    from concourse import bacc, mybir
    from concourse._compat import get_trn_type

    f32 = mybir.dt.float32
    bf16 = mybir.dt.bfloat16

    NSB = 4                # supers per store block (16 groups = 2048 tokens)
    SUP = 4                # groups per super-chunk / psum tile

    nc = bacc.Bacc(get_trn_type() or "TRN2", target_bir_lowering=False, debug=False)
    enc_hi = nc.declare_dram_parameter("enc_hi", [RPC, E, P], bf16, isOutput=False)
    g_hi = nc.declare_dram_parameter("g_hi", [E, E], bf16, isOutput=False)
    pe_w = nc.declare_dram_parameter("pe_w", [128, NGRP, E], bf16, isOutput=False)
    sj_d = nc.declare_dram_parameter("sj", [128, 2 * 128], bf16, isOutput=False)
    # stream rows banked: partitions 32cb..32cb+4 hold [p,p,bt,bt] for token
    # chunk cb (keeps every DMA 128-partition-shaped => cheap in the
    # per-partition-bytes DMA cost model)
    l5_d = nc.declare_dram_parameter("l5", [RPC, 128, T // 4], bf16, isOutput=False)
    w5_d = nc.declare_dram_parameter("w5", [128, E], bf16, isOutput=False)
    out = nc.declare_dram_parameter("out", [RPC, T, E], bf16, isOutput=True)

    with tile.TileContext(nc) as tc, ExitStack() as ctx:
        const = ctx.enter_context(tc.tile_pool(name="const", bufs=1))
        pe_pool = ctx.enter_context(tc.tile_pool(name="pe", bufs=4))
        out_pool = ctx.enter_context(tc.tile_pool(name="outp", bufs=4))
        psum = ctx.enter_context(tc.tile_pool(name="psum", bufs=4, space="PSUM"))

        # SP queue order matters: enc + g first so phase A starts ASAP.
        encT_pool = ctx.enter_context(tc.tile_pool(name="encT", bufs=1))
        ehs = []
        for r in range(RPC):
            eh0 = encT_pool.tile([128, P], bf16, tag=f"eh0_{r}")
            eh1 = encT_pool.tile([128, P], bf16, tag=f"eh1_{r}")
            ehs.append((eh0, eh1))
        gh0 = const.tile([128, E], bf16, tag="gh0")
        gh1 = const.tile([128, E], bf16, tag="gh1")
        nc.sync.dma_start(ehs[0][0][:], enc_hi[0, 0:128, :])
        nc.sync.dma_start(gh0[:], g_hi[0:128, :])
        nc.sync.dma_start(ehs[0][1][:], enc_hi[0, 128:256, :])
        nc.sync.dma_start(gh1[:], g_hi[128:256, :])
        nc.sync.dma_start(ehs[1][0][:], enc_hi[1, 0:128, :])
        nc.sync.dma_start(ehs[1][1][:], enc_hi[1, 128:256, :])
        sj_sb = const.tile([128, 2 * 128], bf16, tag="sj")
        nc.sync.dma_start(sj_sb[:], sj_d[:])
        w5_sb = const.tile([128, E], bf16, tag="w5")
        nc.sync.dma_start(w5_sb[:], w5_d[:])

        l5_sb, egh_keep = [], []
        for r in range(RPC):
            l5_sb.append(
                const.tile([128, T // 4], bf16, tag=f"l5_{r}", name=f"l5_{r}")
            )
            egh_keep.append(
                const.tile([128, 8, E], bf16, tag=f"egh{r}", name=f"egh{r}")
            )
        # token chunk 0 rides the (idle-at-start) Pool queue
        for r in range(RPC):
            nc.gpsimd.dma_start(l5_sb[r][0:4, :], l5_d[r, 0:4, :])

        # ---- phase A: encG = enc @ (I+w_pos); all bf16.
        # psum layout [128p, m, e] == keep layout: row m*128+p at
        # (partition p, block m).  ACT drains psum -> resident bf16.
        for r in range(RPC):
            eh0, eh1 = ehs[r]
            for mh in range(2):
                ps = psum.tile([128, 4 * E], f32, tag="ps")
                for mi in range(4):
                    m = mh * 4 + mi
                    sl = slice(m * 128, (m + 1) * 128)
                    terms = [
                        (eh0, gh0, True, False), (eh1, gh1, False, True),
                    ]
                    for lt, gt_, st, sp in terms:
                        nc.tensor.matmul(
                            ps[:, mi * E:(mi + 1) * E],
                            lhsT=lt[:, sl], rhs=gt_[:], start=st, stop=sp,
                        )
                nc.scalar.copy(
                    egh_keep[r][:, mh * 4:(mh + 1) * 4, :],
                    ps[:].rearrange("p (n e) -> p n e", e=E),
                )

        # remaining stream-row chunks: cb=1 on ACT (after the A drains),
        # cb=2,3 on SP interleaved with the pe loads
        for r in range(RPC):
            nc.scalar.dma_start(l5_sb[r][32:36, :], l5_d[r, 32:36, :])

        # ---- phase B: per 128-token group g, one K=32 one-hot expansion
        # matmul (rows 16q+u of the 32-row window h select encG rows) + one
        # K=4 stream matmul; psum drained per SUP-group super with the pe
        # tile added in.
        for sb in range(NGRP // (NSB * SUP)):          # 4 store blocks
            pe_ts = []
            for half in range(2):
                pt = pe_pool.tile([128, 8, E], bf16, tag="pe")
                g0 = sb * NSB * SUP + half * 8
                nc.sync.dma_start(pt[:], pe_w[:, g0:g0 + 8, :])
                # interleave the remaining stream-row chunk loads on SP
                if sb == 0:
                    cb = 2 + half
                    for r in range(RPC):
                        nc.sync.dma_start(
                            l5_sb[r][32 * cb:32 * cb + 4, :],
                            l5_d[r, 32 * cb:32 * cb + 4, :],
                        )
                pe_ts.append(pt)
            for r in range(RPC):
                ot = out_pool.tile([128, NSB * SUP, E], bf16, tag="ot")
                for ss in range(NSB):
                    ps = psum.tile([128, SUP * E], f32, tag="ps")
                    for j in range(SUP):
                        g = (sb * NSB + ss) * SUP + j
                        h = (g % 8) // 2
                        q = g % 2
                        nc.tensor.matmul(
                            ps[:, j * E:(j + 1) * E],
                            lhsT=sj_sb[32 * h:32 * h + 32, q * 128:(q + 1) * 128],
                            rhs=egh_keep[r][32 * h:32 * h + 32, g // 8, :],
                            start=True, stop=False,
                            tile_position=(32 * h, 0),
                        )
                        cb = g // 16
                        u0 = (g % 16) * 128
                        nc.tensor.matmul(
                            ps[:, j * E:(j + 1) * E],
                            lhsT=l5_sb[r][32 * cb:32 * cb + 4, u0:u0 + 128],
                            rhs=w5_sb[32 * cb:32 * cb + 4, :],
                            start=False, stop=True,
                            tile_position=(32 * cb, 0),
                        )
                    ot_sl = ot[:, ss * SUP:(ss + 1) * SUP, :]
                    pe_sl = pe_ts[ss // 2][:, (ss % 2) * SUP:(ss % 2 + 1) * SUP, :]
                    ps_v = ps[:].rearrange("p (n e) -> p n e", e=E)
                    # drain split tuned so DVE and ACT busy time balance
                    act_drain = (r == 1) or (sb == 3 and ss in (1, 3))
                    if not act_drain:
                        # fused drain: psum f32 + pe bf16 -> bf16 (DVE 1x)
                        nc.vector.tensor_add(ot_sl, ps_v, pe_sl)
                    else:
                        # ACT copy drain, then cheap all-bf16 DVE 2x add
                        nc.scalar.copy(ot_sl, ps_v)
                        nc.vector.tensor_add(ot_sl, ot_sl, pe_sl)
                    # store per 2 supers on the Pool queue (finer tail)
                    if ss % 2 == 1:
                        t0 = (sb * NSB + ss - 1) * SUP * 128
                        nc.gpsimd.dma_start(
                            out[r, t0:t0 + 2 * SUP * 128, :].rearrange(
                                "(n p) e -> p n e", p=128
                            ),
                            ot[:, (ss - 1) * SUP:(ss + 1) * SUP, :],
                        )
    nc.compile()
    return nc


def build_nc_general():
    """Arbitrary-idx path: per-128-token indirect row gathers."""
    import concourse.bass as bass
    from contextlib import ExitStack
    import concourse.tile as tile
    from concourse import bacc, mybir
    from concourse._compat import get_trn_type

    f32 = mybir.dt.float32
    i32 = mybir.dt.int32

    nc = bacc.Bacc(get_trn_type() or "TRN2", target_bir_lowering=False, debug=False)
    enc_t = nc.declare_dram_parameter("enc_t", [RPC, E, P], f32, isOutput=False)
    g_mat = nc.declare_dram_parameter("g_mat", [E, E], f32, isOutput=False)
    pe_w = nc.declare_dram_parameter("pe_w", [128, NGRP, E], f32, isOutput=False)
    p3 = nc.declare_dram_parameter("p3", [RPC, 3, T], f32, isOutput=False)
    w3 = nc.declare_dram_parameter("w3", [3, E], f32, isOutput=False)
    idxo = nc.declare_dram_parameter(
        "idxo", [RPC, 128, NGRP], i32, isOutput=False
    )
    out = nc.declare_dram_parameter("out", [RPC, T, E], f32, isOutput=True)
    encg = nc.dram_tensor("encg", [RPC, P, E], f32)

    with tile.TileContext(nc) as tc, ExitStack() as ctx:
        const = ctx.enter_context(tc.tile_pool(name="const", bufs=1))
        encT_pool = ctx.enter_context(tc.tile_pool(name="encT", bufs=2))
        psum_pool = ctx.enter_context(tc.tile_pool(name="psum", bufs=2, space="PSUM"))
        eg_pool = ctx.enter_context(tc.tile_pool(name="eg", bufs=2))
        pe_pool = ctx.enter_context(tc.tile_pool(name="pe", bufs=2))
        gath_pool = ctx.enter_context(tc.tile_pool(name="gath", bufs=3))

        g0 = const.tile([128, E], f32, tag="g0")
        g1 = const.tile([128, E], f32, tag="g1")
        nc.sync.dma_start(g0[:], g_mat[0:128, :])
        nc.sync.dma_start(g1[:], g_mat[128:256, :])
        w3_sb = const.tile([3, E], f32, tag="w3")
        nc.sync.dma_start(w3_sb[:], w3[:, :])
        p3_sb = []
        ixo_sb = []
        for r in range(RPC):
            p3t = const.tile([3, T], f32, tag=f"p3_{r}")
            nc.sync.dma_start(p3t[:], p3[r])
            p3_sb.append(p3t)
            ixt = const.tile([128, NGRP], i32, tag=f"ixo_{r}")
            nc.sync.dma_start(ixt[:], idxo[r])
            ixo_sb.append(ixt)

        for r in range(RPC):
            et0 = encT_pool.tile([128, P], f32, tag="et0")
            et1 = encT_pool.tile([128, P], f32, tag="et1")
            nc.sync.dma_start(et0[:], enc_t[r, 0:128, :])
            nc.sync.dma_start(et1[:], enc_t[r, 128:256, :])
            ps = psum_pool.tile([128, 8 * E], f32, tag="ps")
            for m in range(8):
                nc.tensor.matmul(
                    ps[:, m * E:(m + 1) * E],
                    lhsT=et0[:, m * 128:(m + 1) * 128],
                    rhs=g0[:], start=True, stop=False,
                )
                nc.tensor.matmul(
                    ps[:, m * E:(m + 1) * E],
                    lhsT=et1[:, m * 128:(m + 1) * 128],
                    rhs=g1[:], start=False, stop=True,
                )
            eg = eg_pool.tile([128, 8 * E], f32, tag="eg")
            nc.vector.tensor_copy(eg[:], ps[:])
            nc.sync.dma_start(
                encg[r].rearrange("(m p) e -> p m e", p=128),
                eg[:].rearrange("q (m e) -> q m e", e=E),
            )

        encg_flat = encg[:].rearrange("r p e -> (r p) e")
        for s in range(T // (NSUP * 128)):
            pe_t = pe_pool.tile([128, NSUP, E], f32, tag="pe")
            nc.sync.dma_start(pe_t[:], pe_w[:, s * NSUP:(s + 1) * NSUP, :])
            for r in range(RPC):
                gt = gath_pool.tile([128, NSUP, E], f32, tag="gt")
                for g in range(NSUP):
                    gi = s * NSUP + g
                    nc.gpsimd.indirect_dma_start(
                        out=gt[:, g, :],
                        out_offset=None,
                        in_=encg_flat,
                        in_offset=bass.IndirectOffsetOnAxis(
                            ap=ixo_sb[r][:, gi:gi + 1], axis=0
                        ),
                    )
                nc.vector.tensor_add(gt[:], gt[:], pe_t[:])
                ps = psum_pool.tile([128, 8 * E], f32, tag="ps")
                for g in range(NSUP):
                    gi = s * NSUP + g
                    nc.tensor.matmul(
                        ps[:, g * E:(g + 1) * E],
                        lhsT=p3_sb[r][:, gi * 128:(gi + 1) * 128],
                        rhs=w3_sb[:],
                        start=True, stop=True,
                    )
                nc.vector.tensor_add(
                    gt[:], gt[:], ps[:].rearrange("q (n e) -> q n e", e=E)
                )
                nc.sync.dma_start(
                    out[r, s * NSUP * 128:(s + 1) * NSUP * 128, :].rearrange(
                        "(n p) e -> p n e", p=128
                    ),
                    gt[:],
                )
    nc.compile()
    return nc


def get_nc(fast):
    key = "nc_fast" if fast else "nc_gen"
    if key not in _CACHE:
        _CACHE[key] = build_nc_fast() if fast else build_nc_general()
    return _CACHE[key]


# --------------------------------------------------------------------------
# Host wrapper
# --------------------------------------------------------------------------
def make_in_maps(encoder_out, align_phone, text_phone, pitch, beats,
                 w_pitch, b_pitch, emb_beats, w_pos, b_pos):
    import ml_dtypes

    encoder_out = np.asarray(encoder_out, np.float32)
    pitch = np.asarray(pitch, np.float32)
    beats = np.asarray(beats)
    w_pitch = np.asarray(w_pitch, np.float32)
    w_pos = np.asarray(w_pos, np.float32)

    idx = compute_idx(np.asarray(align_phone), np.asarray(text_phone))
    fast = bool(np.all(idx == (np.arange(T, dtype=np.int32) // DUR)[None, :]))
    if FORCE_GENERAL:
        fast = False

    g_mat = (np.eye(E, dtype=np.float64) + w_pos.astype(np.float64)).astype(np.float32)
    pe = _positional_encoding_f64(T, E)
    pe_proj = pe @ w_pos.astype(np.float64)                          # [T, E]
    bias = (np.asarray(emb_beats[0], np.float64)
            + np.asarray(b_pitch, np.float64)
            + np.asarray(b_pos, np.float64))
    demb = (np.asarray(emb_beats[1], np.float64)
            - np.asarray(emb_beats[0], np.float64)).astype(np.float32)

    if fast:
        pe_tot = (pe_proj + bias[None, :]).astype(ml_dtypes.bfloat16)
        pe_wrap = np.ascontiguousarray(pe_tot.reshape(NGRP, 128, E).swapaxes(0, 1))
        # sj[p, q*128+dt] = 1 iff (p%32) == 16q + dt//8   (K=32 windows)
        rows = np.arange(128)[:, None] % 32
        sj = np.concatenate(
            [(rows == (q * NW + np.arange(128) // DUR)[None, :]) for q in range(2)],
            axis=1,
        ).astype(ml_dtypes.bfloat16)
        g_hi = g_mat.astype(ml_dtypes.bfloat16)
        wp_hi, wp_lo = _bf16_split(w_pitch[0])
        db_hi, db_lo = _bf16_split(demb)
        w5_rows = np.stack([wp_hi, wp_lo, db_hi, db_lo]).astype(ml_dtypes.bfloat16)
        w5 = np.zeros((128, E), ml_dtypes.bfloat16)
        for cb in range(4):
            w5[32 * cb:32 * cb + 4] = w5_rows
        fast_common = {
            "pe_w": pe_wrap, "sj": sj,
            "g_hi": g_hi, "w5": w5,
        }
    else:
        w3 = np.stack(
            [w_pitch[0].astype(np.float64), demb.astype(np.float64), bias]
        ).astype(np.float32)
        pe_wl = np.ascontiguousarray(
            pe_proj.astype(np.float32).reshape(NGRP, 128, E).swapaxes(0, 1)
        )

    in_maps = []
    for core in range(NCORES):
        rows_ = range(core * RPC, (core + 1) * RPC)
        enc_t = np.ascontiguousarray(
            encoder_out[core * RPC:(core + 1) * RPC].transpose(0, 2, 1)
        )
        if fast:
            import ml_dtypes as _md
            enc_hi = enc_t.astype(_md.bfloat16)
            l5 = np.zeros((RPC, 128, T // 4), _md.bfloat16)
            for j, b in enumerate(rows_):
                p_hi = pitch[b, :, 0].astype(_md.bfloat16)
                bt = beats[b, :, 0].astype(_md.bfloat16)
                rows4 = np.stack([p_hi, p_hi, bt, bt])       # [4, T]
                for cb in range(4):
                    l5[j, 32 * cb:32 * cb + 4] = rows4[
                        :, cb * (T // 4):(cb + 1) * (T // 4)
                    ]
            m = {"enc_hi": enc_hi, "l5": l5, **fast_common}
        else:
            p3 = np.empty((RPC, 3, T), np.float32)
            idxo = np.empty((RPC, 128, NGRP), np.int32)
            for j, b in enumerate(rows_):
                p3[j, 0] = pitch[b, :, 0]
                p3[j, 1] = beats[b, :, 0].astype(np.float32)
                p3[j, 2] = 1.0
                idxo[j] = idx[b].reshape(NGRP, 128).T + j * P
            m = {"enc_t": enc_t, "g_mat": g_mat, "pe_w": pe_wl, "p3": p3,
                 "w3": w3, "idxo": idxo}
        in_maps.append(m)
    return fast, in_maps


def kernel(**inputs):
    from concourse.bass_utils import run_bass_kernel_spmd

    fast, in_maps = make_in_maps(**inputs)
    nc = get_nc(fast)
    res = run_bass_kernel_spmd(nc, in_maps, core_ids=list(range(NCORES)))
    out = np.concatenate([res.results[i]["out"] for i in range(NCORES)], axis=0)
    return np.ascontiguousarray(out.astype(np.float32))


# revision 16
# speedup vs baseline: 2.2104x; 1.2016x over previous
"""Trainium2 Bass kernel for nn_Encoder_Postnet (ragged_sequence).

Computation (reference):
    idx   = sequential aligner scan over (align_phone, text_phone)   [B,T]
    out   = enc[idx] + pitch @ w_pitch + b_pitch + emb_beats[beats]
            + (enc[idx] + pe) @ w_pos + b_pos

Key algebraic restructure: the duration-expansion gather commutes with the
E x E linear, so
    out[t] = encG[idx_t] + (pe@w_pos + bias)[t] + pitch[t]*w_pitch + beats[t]*demb
with encG = enc @ (I + w_pos) computed once per batch row ([P,E] not [T,E]),
collapsing the big [B*T,E]@[E,E] matmul 8x and making the kernel memory-bound.

Sharding: pure data parallel, 2 batch rows per core across 8 cores.

Fast path (the uniform duration-8 expansion this model's inputs produce,
idx == arange(T)//8 for every row).  The whole dataflow is bf16 (the harness
tolerance is 2e-2 relmax; bf16 roundoff is ~4e-3), which halves every DMA
stream vs f32 -- the cost model serializes all DMA traffic at 360 GB/s so
bytes moved is the wall clock:
  phase A: encG = enc_bf16 @ (I+w_pos) on PE (g kept as bf16 hi+lo for
           accuracy); psum drained by the ACT engine to a resident bf16 tile.
  phase B: per 128-token group, one K=32 one-hot expansion matmul (selects
           and repeats the 16 encG rows for this group) + one K=4 stream
           matmul (pitch x w_pitch hi/lo + beats x demb hi/lo) into PSUM.
           Per 4-group super-chunk the psum is drained with the
           (pe@w_pos + bias) bf16 tile added in:
             row 0: DVE fused tensor_add (psum + pe -> bf16 out tile)
             row 1: ACT copy (psum -> bf16) then DVE 2x-mode bf16 add of pe
           which balances DVE/ACT busy time under the DMA roofline.
  Output is stored as bf16 (converted to f32 on host).

General path (arbitrary idx): per-128-token indirect-DMA row gathers
(production-shaped offset [128,1] DynamicAP descriptors) + K=3 stream matmul.

The aligner scan itself is index metadata ([B,T] int32); it is computed on
host with a run-compressed O(B*P) algorithm exactly equivalent to the
reference recurrence, then consumed either as a uniformity proof (fast path)
or as gather offsets (general path).
"""

import sys

for _p in ("/opt/trn_rl_repo",):
    if _p not in sys.path:
        sys.path.insert(0, _p)

import numpy as np

B, P, T, E = 16, 1024, 8192, 256
NCORES = 8
RPC = B // NCORES          # batch rows per core
NGRP = T // 128            # 64 groups of 128 tokens per row
NSUP = 8                   # groups per super-chunk (general path)
DUR = T // P               # uniform duration of the fast path (8)
NW = 128 // DUR            # encG rows per group (16)

FORCE_GENERAL = False      # test hook: force the arbitrary-idx path
_CACHE = {}


# --------------------------------------------------------------------------
# Host: aligner index computation (exact replica of the reference recurrence)
# --------------------------------------------------------------------------
def compute_idx(align, text):
    """idx[b,0]=0; idx[b,j] = idx[b,j-1] if align[b,j]==text[b,idx[b,j-1]]
    else min(idx[b,j-1]+1, P-1).   Vectorized over batch via segment starts:
    the pointer advances i->i+1 at s_{i+1} = first j >= s_i+1 with
    align[j] != text[i]; within a run of align values equal to text[i] the
    first mismatch is the run end."""
    align = np.asarray(align)
    text = np.asarray(text)
    Bn, Tn = align.shape
    Pn = text.shape[1]
    diff = align[:, 1:] != align[:, :-1]                       # [B, T-1]
    c = np.full((Bn, Tn), Tn, np.int64)
    c[:, :-1] = np.where(diff, np.arange(1, Tn)[None, :], Tn)
    re = np.flip(np.minimum.accumulate(np.flip(c, axis=1), axis=1), axis=1)

    s = np.full((Bn, Pn), Tn, np.int64)
    s[:, 0] = 0
    cur = np.zeros(Bn, np.int64)
    arB = np.arange(Bn)
    for i in range(Pn - 1):
        j0 = cur + 1
        active = j0 < Tn
        j0c = np.minimum(j0, Tn - 1)
        eq = (align[arB, j0c] == text[:, i]) & active
        nxt = np.where(active, np.where(eq, re[arB, j0c], j0), Tn)
        s[:, i + 1] = nxt
        cur = nxt
    idx = np.empty((Bn, Tn), np.int32)
    pos = np.arange(Tn)
    for b in range(Bn):
        idx[b] = (np.searchsorted(s[b], pos, side="right") - 1).astype(np.int32)
    return idx


def _positional_encoding_f64(t, e):
    pos = np.arange(t, dtype=np.float64)[:, None]
    div = np.exp(np.arange(0, e, 2, dtype=np.float64) * (-np.log(10000.0) / e))
    ang = pos * div[None, :]
    return np.stack([np.sin(ang), np.cos(ang)], axis=-1).reshape(t, e)


def _bf16_split(x):
    import ml_dtypes
    x = np.asarray(x, np.float32)
    hi = x.astype(ml_dtypes.bfloat16)
    lo = (x - hi.astype(np.float32)).astype(ml_dtypes.bfloat16)
    return hi, lo


def _fp8_split(x):
    import ml_dtypes
    x = np.asarray(x, np.float32)
    hi = x.astype(ml_dtypes.float8_e4m3)
    lo = (x - hi.astype(np.float32)).astype(ml_dtypes.float8_e4m3)
    return hi, lo


# --------------------------------------------------------------------------
# Device programs
# --------------------------------------------------------------------------
def build_nc_fast():
    from contextlib import ExitStack
    import concourse.tile as tile
    from concourse import bacc, mybir
    from concourse._compat import get_trn_type

    f32 = mybir.dt.float32
    bf16 = mybir.dt.bfloat16
    f8 = mybir.dt.float8e4
    DR = mybir.MatmulPerfMode.DoubleRow
    mult = mybir.AluOpType.mult
    add = mybir.AluOpType.add

    NSB = 4                # supers per store block (16 groups = 2048 tokens)
    SUP = 4                # groups per super-chunk / psum tile

    nc = bacc.Bacc(get_trn_type() or "TRN2", target_bir_lowering=False, debug=False)
    enc_hi = nc.declare_dram_parameter("enc_hi", [RPC, E, P], bf16, isOutput=False)
    g_hi = nc.declare_dram_parameter("g_hi", [E, E], bf16, isOutput=False)
    pe_w = nc.declare_dram_parameter("pe_w", [128, NGRP, E], bf16, isOutput=False)
    # one-hot expansion selectors for fp8 DoubleRow (hi|lo pair blocks)
    sj_d = nc.declare_dram_parameter("sj", [128, 2, 2, 128], f8, isOutput=False)
    # stream rows banked: partitions 32cb..32cb+3 hold the DoubleRow pairs
    # (p_hi|p_hi), (p_lo|0), (bt|bt) for token chunk cb
    l5_d = nc.declare_dram_parameter("l5", [RPC, 128, 2, T // 4], f8, isOutput=False)
    w5_d = nc.declare_dram_parameter("w5", [128, 2, E], f8, isOutput=False)
    out = nc.declare_dram_parameter("out", [RPC, T, E], bf16, isOutput=True)

    with tile.TileContext(nc) as tc, ExitStack() as ctx:
        const = ctx.enter_context(tc.tile_pool(name="const", bufs=1))
        pe_pool = ctx.enter_context(tc.tile_pool(name="pe", bufs=4))
        out_pool = ctx.enter_context(tc.tile_pool(name="outp", bufs=6))
        psum = ctx.enter_context(tc.tile_pool(name="psum", bufs=4, space="PSUM"))

        # Queue plan (v1 cost model: DMA transfer time occupies the issuing
        # queue; Pool tensor ops are the cheapest drain at 0.83ns/row):
        #   SP  : enc, pe 0-5, l5 cb3, 8 stores
        #   DVE : g/sj/w5 consts, l5 cb2(r1), 12 fused drains, 8 stores
        #   ACT : l5 cb0/cb1/cb2(r0), pe 6-7, 16 stores
        #   Pool: phase-A hi/lo drains + 20 fused drains (no DMA)
        #   PE  : matmuls only
        encT_pool = ctx.enter_context(tc.tile_pool(name="encT", bufs=1))
        ehs = []
        for r in range(RPC):
            eh0 = encT_pool.tile([128, P], bf16, tag=f"eh0_{r}")
            eh1 = encT_pool.tile([128, P], bf16, tag=f"eh1_{r}")
            ehs.append((eh0, eh1))
        gh0 = const.tile([128, E], bf16, tag="gh0")
        gh1 = const.tile([128, E], bf16, tag="gh1")
        sj_sb = const.tile([128, 2, 2, 128], f8, tag="sj")
        w5_sb = const.tile([128, 2, E], f8, tag="w5")
        nc.sync.dma_start(ehs[0][0][:], enc_hi[0, 0:128, :])
        nc.sync.dma_start(gh0[:], g_hi[0:128, :])
        nc.sync.dma_start(ehs[0][1][:], enc_hi[0, 128:256, :])
        nc.sync.dma_start(gh1[:], g_hi[128:256, :])
        nc.sync.dma_start(ehs[1][0][:], enc_hi[1, 0:128, :])
        nc.sync.dma_start(ehs[1][1][:], enc_hi[1, 128:256, :])
        nc.sync.dma_start(sj_sb[:], sj_d[:])
        nc.sync.dma_start(w5_sb[:], w5_d[:])

        l5_sb, egh_keep = [], []
        for r in range(RPC):
            l5_sb.append(
                const.tile([128, 2, T // 4], f8, tag=f"l5_{r}", name=f"l5_{r}")
            )
            egh_keep.append(
                const.tile([128, 8, 2, E], f8, tag=f"egh{r}", name=f"egh{r}")
            )
        for cb in range(2):
            for r in range(RPC):
                nc.scalar.dma_start(
                    l5_sb[r][32 * cb:32 * cb + 3, :, :],
                    l5_d[r, 32 * cb:32 * cb + 3, :, :],
                )
        nc.gpsimd.dma_start(l5_sb[0][96:99, :, :], l5_d[0, 96:99, :, :])

        # ---- phase A: encG = enc @ (I+w_pos) in bf16; Pool drains the psum
        # into resident fp8 hi|lo pair blocks for the DoubleRow expansion.
        for r in range(RPC):
            eh0, eh1 = ehs[r]
            for mh in range(2):
                ps = psum.tile([128, 4 * E], f32, tag="ps")
                for mi in range(4):
                    m = mh * 4 + mi
                    sl = slice(m * 128, (m + 1) * 128)
                    nc.tensor.matmul(
                        ps[:, mi * E:(mi + 1) * E],
                        lhsT=eh0[:, sl], rhs=gh0[:], start=True, stop=False,
                    )
                    nc.tensor.matmul(
                        ps[:, mi * E:(mi + 1) * E],
                        lhsT=eh1[:, sl], rhs=gh1[:], start=False, stop=True,
                    )
                ps_v = ps[:].rearrange("p (n e) -> p n e", e=E)
                hi_sl = egh_keep[r][:, mh * 4:(mh + 1) * 4, 0, :]
                nc.gpsimd.tensor_copy(hi_sl, ps_v)
                nc.gpsimd.scalar_tensor_tensor(
                    out=egh_keep[r][:, mh * 4:(mh + 1) * 4, 1, :],
                    in0=hi_sl, scalar=-1.0, in1=ps_v, op0=mult, op1=add,
                )

        nc.scalar.dma_start(l5_sb[0][64:67, :, :], l5_d[0, 64:67, :, :])
        nc.scalar.dma_start(l5_sb[1][64:67, :, :], l5_d[1, 64:67, :, :])

        # ---- phase B: per 128-token group, one fp8 DoubleRow one-hot
        # expansion matmul (hi+lo in a single pass) + one DoubleRow stream
        # matmul; psum drained per SUP-group super with the bf16 pe tile
        # added in (fused tensor_add on Pool/DVE).
        didx = 0
        sidx = 0
        STORE_Q = [2, 0, 2, 2, 0, 2, 0, 2] * 4   # 0=SP 2=ACT
        for sb in range(NGRP // (NSB * SUP)):          # 4 store blocks
            pe_ts = []
            for half in range(2):
                pt = pe_pool.tile([128, 8, E], bf16, tag="pe")
                g0 = sb * NSB * SUP + half * 8
                eng = nc.gpsimd if sb == 3 else nc.sync
                eng.dma_start(pt[:], pe_w[:, g0:g0 + 8, :])
                pe_ts.append(pt)
            if sb == 1:
                nc.gpsimd.dma_start(l5_sb[1][96:99, :, :], l5_d[1, 96:99, :, :])
            for r in range(RPC):
                for ss in range(NSB):
                    ps = psum.tile([128, SUP * E], f32, tag="ps")
                    for j in range(SUP):
                        g = (sb * NSB + ss) * SUP + j
                        h = (g % 8) // 2
                        q = g % 2
                        nc.tensor.matmul(
                            ps[:, j * E:(j + 1) * E],
                            lhsT=sj_sb[32 * h:32 * h + 32, q, :, :],
                            rhs=egh_keep[r][32 * h:32 * h + 32, g // 8, :, :],
                            start=True, stop=False, perf_mode=DR,
                            tile_position=(32 * h, 0),
                        )
                        cb = g // 16
                        u0 = (g % 16) * 128
                        nc.tensor.matmul(
                            ps[:, j * E:(j + 1) * E],
                            lhsT=l5_sb[r][32 * cb:32 * cb + 3, :, u0:u0 + 128],
                            rhs=w5_sb[32 * cb:32 * cb + 3, :, :],
                            start=False, stop=True, perf_mode=DR,
                            tile_position=(32 * cb, 0),
                        )
                    ot = out_pool.tile([128, SUP, E], bf16, tag="ot")
                    pe_sl = pe_ts[ss // 2][:, (ss % 2) * SUP:(ss % 2 + 1) * SUP, :]
                    ps_v = ps[:].rearrange("p (n e) -> p n e", e=E)
                    # fused drain+pe-add: Pool for 20 supers, DVE for 12
                    eng = nc.gpsimd if didx % 8 in (0, 3, 6) else nc.vector
                    eng.tensor_add(ot[:], ps_v, pe_sl)
                    didx += 1
                    t0 = (sb * NSB + ss) * SUP * 128
                    st_eng = (nc.sync, nc.gpsimd, nc.scalar)[STORE_Q[sidx]]
                    sidx += 1
                    st_eng.dma_start(
                        out[r, t0:t0 + SUP * 128, :].rearrange(
                            "(n p) e -> p n e", p=128
                        ),
                        ot[:],
                    )
    nc.compile()
    return nc


def build_nc_general():
    """Arbitrary-idx path: per-128-token indirect row gathers."""
    import concourse.bass as bass
    from contextlib import ExitStack
    import concourse.tile as tile
    from concourse import bacc, mybir
    from concourse._compat import get_trn_type

    f32 = mybir.dt.float32
    i32 = mybir.dt.int32

    nc = bacc.Bacc(get_trn_type() or "TRN2", target_bir_lowering=False, debug=False)
    enc_t = nc.declare_dram_parameter("enc_t", [RPC, E, P], f32, isOutput=False)
    g_mat = nc.declare_dram_parameter("g_mat", [E, E], f32, isOutput=False)
    pe_w = nc.declare_dram_parameter("pe_w", [128, NGRP, E], f32, isOutput=False)
    p3 = nc.declare_dram_parameter("p3", [RPC, 3, T], f32, isOutput=False)
    w3 = nc.declare_dram_parameter("w3", [3, E], f32, isOutput=False)
    idxo = nc.declare_dram_parameter(
        "idxo", [RPC, 128, NGRP], i32, isOutput=False
    )
    out = nc.declare_dram_parameter("out", [RPC, T, E], f32, isOutput=True)
    encg = nc.dram_tensor("encg", [RPC, P, E], f32)

    with tile.TileContext(nc) as tc, ExitStack() as ctx:
        const = ctx.enter_context(tc.tile_pool(name="const", bufs=1))
        encT_pool = ctx.enter_context(tc.tile_pool(name="encT", bufs=2))
        psum_pool = ctx.enter_context(tc.tile_pool(name="psum", bufs=2, space="PSUM"))
        eg_pool = ctx.enter_context(tc.tile_pool(name="eg", bufs=2))
        pe_pool = ctx.enter_context(tc.tile_pool(name="pe", bufs=2))
        gath_pool = ctx.enter_context(tc.tile_pool(name="gath", bufs=3))

        g0 = const.tile([128, E], f32, tag="g0")
        g1 = const.tile([128, E], f32, tag="g1")
        nc.sync.dma_start(g0[:], g_mat[0:128, :])
        nc.sync.dma_start(g1[:], g_mat[128:256, :])
        w3_sb = const.tile([3, E], f32, tag="w3")
        nc.sync.dma_start(w3_sb[:], w3[:, :])
        p3_sb = []
        ixo_sb = []
        for r in range(RPC):
            p3t = const.tile([3, T], f32, tag=f"p3_{r}")
            nc.sync.dma_start(p3t[:], p3[r])
            p3_sb.append(p3t)
            ixt = const.tile([128, NGRP], i32, tag=f"ixo_{r}")
            nc.sync.dma_start(ixt[:], idxo[r])
            ixo_sb.append(ixt)

        for r in range(RPC):
            et0 = encT_pool.tile([128, P], f32, tag="et0")
            et1 = encT_pool.tile([128, P], f32, tag="et1")
            nc.sync.dma_start(et0[:], enc_t[r, 0:128, :])
            nc.sync.dma_start(et1[:], enc_t[r, 128:256, :])
            ps = psum_pool.tile([128, 8 * E], f32, tag="ps")
            for m in range(8):
                nc.tensor.matmul(
                    ps[:, m * E:(m + 1) * E],
                    lhsT=et0[:, m * 128:(m + 1) * 128],
                    rhs=g0[:], start=True, stop=False,
                )
                nc.tensor.matmul(
                    ps[:, m * E:(m + 1) * E],
                    lhsT=et1[:, m * 128:(m + 1) * 128],
                    rhs=g1[:], start=False, stop=True,
                )
            eg = eg_pool.tile([128, 8 * E], f32, tag="eg")
            nc.vector.tensor_copy(eg[:], ps[:])
            nc.sync.dma_start(
                encg[r].rearrange("(m p) e -> p m e", p=128),
                eg[:].rearrange("q (m e) -> q m e", e=E),
            )

        encg_flat = encg[:].rearrange("r p e -> (r p) e")
        for s in range(T // (NSUP * 128)):
            pe_t = pe_pool.tile([128, NSUP, E], f32, tag="pe")
            nc.sync.dma_start(pe_t[:], pe_w[:, s * NSUP:(s + 1) * NSUP, :])
            for r in range(RPC):
                gt = gath_pool.tile([128, NSUP, E], f32, tag="gt")
                for g in range(NSUP):
                    gi = s * NSUP + g
                    nc.gpsimd.indirect_dma_start(
                        out=gt[:, g, :],
                        out_offset=None,
                        in_=encg_flat,
                        in_offset=bass.IndirectOffsetOnAxis(
                            ap=ixo_sb[r][:, gi:gi + 1], axis=0
                        ),
                    )
                nc.vector.tensor_add(gt[:], gt[:], pe_t[:])
                ps = psum_pool.tile([128, 8 * E], f32, tag="ps")
                for g in range(NSUP):
                    gi = s * NSUP + g
                    nc.tensor.matmul(
                        ps[:, g * E:(g + 1) * E],
                        lhsT=p3_sb[r][:, gi * 128:(gi + 1) * 128],
                        rhs=w3_sb[:],
                        start=True, stop=True,
                    )
                nc.vector.tensor_add(
                    gt[:], gt[:], ps[:].rearrange("q (n e) -> q n e", e=E)
                )
                nc.sync.dma_start(
                    out[r, s * NSUP * 128:(s + 1) * NSUP * 128, :].rearrange(
                        "(n p) e -> p n e", p=128
                    ),
                    gt[:],
                )
    nc.compile()
    return nc


def get_nc(fast):
    key = "nc_fast" if fast else "nc_gen"
    if key not in _CACHE:
        _CACHE[key] = build_nc_fast() if fast else build_nc_general()
    return _CACHE[key]


# --------------------------------------------------------------------------
# Host wrapper
# --------------------------------------------------------------------------
def make_in_maps(encoder_out, align_phone, text_phone, pitch, beats,
                 w_pitch, b_pitch, emb_beats, w_pos, b_pos):
    import ml_dtypes

    encoder_out = np.asarray(encoder_out, np.float32)
    pitch = np.asarray(pitch, np.float32)
    beats = np.asarray(beats)
    w_pitch = np.asarray(w_pitch, np.float32)
    w_pos = np.asarray(w_pos, np.float32)

    idx = compute_idx(np.asarray(align_phone), np.asarray(text_phone))
    fast = bool(np.all(idx == (np.arange(T, dtype=np.int32) // DUR)[None, :]))
    if FORCE_GENERAL:
        fast = False

    g_mat = (np.eye(E, dtype=np.float64) + w_pos.astype(np.float64)).astype(np.float32)
    pe = _positional_encoding_f64(T, E)
    pe_proj = pe @ w_pos.astype(np.float64)                          # [T, E]
    bias = (np.asarray(emb_beats[0], np.float64)
            + np.asarray(b_pitch, np.float64)
            + np.asarray(b_pos, np.float64))
    demb = (np.asarray(emb_beats[1], np.float64)
            - np.asarray(emb_beats[0], np.float64)).astype(np.float32)

    if fast:
        pe_tot = (pe_proj + bias[None, :]).astype(ml_dtypes.bfloat16)
        pe_wrap = np.ascontiguousarray(pe_tot.reshape(NGRP, 128, E).swapaxes(0, 1))
        # sj[p, q, i, dt] = 1 iff (p%32) == 16q + dt//8  (both pair blocks i)
        rows = np.arange(128)[:, None] % 32
        sj = np.zeros((128, 2, 2, 128), ml_dtypes.float8_e4m3)
        for q in range(2):
            oh = (rows == (q * NW + np.arange(128) // DUR)[None, :])
            sj[:, q, 0] = oh
            sj[:, q, 1] = oh
        g_hi = g_mat.astype(ml_dtypes.bfloat16)
        f8t = ml_dtypes.float8_e4m3
        wp_hi, wp_lo = _fp8_split(w_pitch[0])
        db_hi, db_lo = _fp8_split(demb)
        # w5 DoubleRow pair blocks: (w_hi|w_lo), (w_hi|0), (db_hi|db_lo)
        w5 = np.zeros((128, 2, E), f8t)
        for cb in range(4):
            w5[32 * cb + 0, 0] = wp_hi
            w5[32 * cb + 0, 1] = wp_lo
            w5[32 * cb + 1, 0] = wp_hi
            w5[32 * cb + 2, 0] = db_hi
            w5[32 * cb + 2, 1] = db_lo
        fast_common = {
            "pe_w": pe_wrap, "sj": sj,
            "g_hi": g_hi, "w5": w5,
        }
    else:
        w3 = np.stack(
            [w_pitch[0].astype(np.float64), demb.astype(np.float64), bias]
        ).astype(np.float32)
        pe_wl = np.ascontiguousarray(
            pe_proj.astype(np.float32).reshape(NGRP, 128, E).swapaxes(0, 1)
        )

    in_maps = []
    for core in range(NCORES):
        rows_ = range(core * RPC, (core + 1) * RPC)
        enc_t = np.ascontiguousarray(
            encoder_out[core * RPC:(core + 1) * RPC].transpose(0, 2, 1)
        )
        if fast:
            import ml_dtypes as _md
            enc_hi = enc_t.astype(_md.bfloat16)
            l5 = np.zeros((RPC, 128, 2, T // 4), _md.float8_e4m3)
            for j, b in enumerate(rows_):
                p_hi, p_lo = _fp8_split(pitch[b, :, 0])
                bt = beats[b, :, 0].astype(_md.float8_e4m3)
                for cb in range(4):
                    s_ = slice(cb * (T // 4), (cb + 1) * (T // 4))
                    l5[j, 32 * cb + 0, 0] = p_hi[s_]
                    l5[j, 32 * cb + 0, 1] = p_hi[s_]
                    l5[j, 32 * cb + 1, 0] = p_lo[s_]
                    l5[j, 32 * cb + 2, 0] = bt[s_]
                    l5[j, 32 * cb + 2, 1] = bt[s_]
            m = {"enc_hi": enc_hi, "l5": l5, **fast_common}
        else:
            p3 = np.empty((RPC, 3, T), np.float32)
            idxo = np.empty((RPC, 128, NGRP), np.int32)
            for j, b in enumerate(rows_):
                p3[j, 0] = pitch[b, :, 0]
                p3[j, 1] = beats[b, :, 0].astype(np.float32)
                p3[j, 2] = 1.0
                idxo[j] = idx[b].reshape(NGRP, 128).T + j * P
            m = {"enc_t": enc_t, "g_mat": g_mat, "pe_w": pe_wl, "p3": p3,
                 "w3": w3, "idxo": idxo}
        in_maps.append(m)
    return fast, in_maps


def kernel(**inputs):
    from concourse.bass_utils import run_bass_kernel_spmd

    fast, in_maps = make_in_maps(**inputs)
    nc = get_nc(fast)
    res = run_bass_kernel_spmd(nc, in_maps, core_ids=list(range(NCORES)))
    out = np.concatenate([res.results[i]["out"] for i in range(NCORES)], axis=0)
    return np.ascontiguousarray(out.astype(np.float32))
